# revision 41
# baseline (speedup 1.0000x reference)
"""Multi-head attention (B=8, N=1024, C=768, H=12) on 8 TRN2 NeuronCores.

Sharding: pure data-parallel over batch - core b computes attention for x[b].
Per-core Bass/Tile kernel, bf16 compute, f32 PSUM.

v3 schedule (orientation-B O):
  qkv/S unchanged from v2: qkv psum pairs heads on partition halves; S
  matmuls per (mt, nch) write [128 keys, 2 heads, 512 q] PSUM; one exp per
  (mt, nch) covers both heads -> E_ab[128, mt, nch, ab, 512] bf16.

  O restructured: out[q, d] = E_chunk^T @ v with E as the (free) stationary
  operand and v [128, 65] moving (ones col -> softmax sums in col 64).
  Per (head, qtile): 8 accumulating matmuls of 65 columns instead of the
  old [65, 512] orientation - halves the O column count on the PE.

  norm: DVE reciprocal of the PSUM sums column + per-partition
  tensor_scalar_mul -> normalized ob[q, d] bf16 in SBUF.

  transpose: proj needs on[d, q]; ob^T comes from a PE matmul against an
  identity matrix (ldweights are free): ot[64, 128] = ob_qt^T @ I, then one
  DVE copy per half moves [64, 4, 128] PSUM -> on_sb.

  PSUM budget (16KB/partition): tag "s" 2x[128,2,512]f32 (S double-buffer,
  also startup qk pair-0), tag "o" 2x[128,4,128]f32 (O accumulator halves
  and transpose outputs alternate through the same two slots), tag "f"
  1x4KB (warmup, qk/v/proj fillers, serial by construction).

  Per phase hp (S/exp of pair hp, O of pair hp-1), chains spread over mts:
    mt0 a-lo qt01 | mt1 a-lo qt23 +norm | mt2 b-lo qt01 | mt3 b-lo qt23
    +norm, T(a-lo)+copy | mt4 T(b-lo)+copy, a-hi qt45 | mt5 a-hi qt67
    +norm | mt6 b-hi qt45 | mt7 b-hi qt67 +norm, T(a-hi)+copy, T(b-hi)+copy
  so each "o" slot's next allocation waits only on work finished ~2 mts ago.

  tail: O(pair 5) same pattern dense; proj kt0-3 partials prestaged as
  phase-4/5 fillers (DVE stage to SBUF), tail does kt4-5 + fused epilogue
  (partial + psum + bias on DVE) + split-queue output DMA.
"""

import numpy as np
import ml_dtypes

B, N, C = 8, 1024, 768
H, D = 12, 64
SCALE = D ** -0.5
CT = C // 128        # 6 contraction tiles
NT = N // 128        # 8 token tiles
NCH = N // 512       # 2 n-chunks of 512
HP = H // 2          # 6 head pairs

_CACHE = {}


def _build_nc(loop_r=None):
    import concourse.bacc as bacc
    import concourse.mybir as mybir
    import concourse.tile as tile

    f32 = mybir.dt.float32
    bf16 = mybir.dt.bfloat16

    nc = bacc.Bacc("TRN2", target_bir_lowering=False, debug=False, num_devices=8)

    xT_d = nc.dram_tensor("xT", [C, N], bf16, kind="ExternalInput").ap()
    # weights declared row-tiled [CT, 128, ...] so one multi-descriptor DMA
    # (single HWDGE issue) can stage a whole weight with dst partitions =
    # the inner 128 rows
    # wqk layout [role, pair, row128, kt, d]: per-(role, pair-slice) DMA
    # opts to 3 dims (row, pair, kt*d) on both sides
    wqk_d = nc.dram_tensor("wqk", [2, CT, 128, CT, 128], bf16,
                           kind="ExternalInput").ap()
    wv_d = nc.dram_tensor("wv", [CT, 128, C], bf16, kind="ExternalInput").ap()
    wp_d = nc.dram_tensor("wp", [CT, 128, C], bf16, kind="ExternalInput").ap()
    pb_d = nc.dram_tensor("pb", [128, CT], f32, kind="ExternalInput").ap()
    eye_d = nc.dram_tensor("eye", [128, 128], bf16, kind="ExternalInput").ap()
    out_d = nc.dram_tensor("out", [C, N], bf16, kind="ExternalOutput").ap()

    with tile.TileContext(nc) as tc:
        with (
            tc.tile_pool(name="const", bufs=1) as cpool,
            tc.tile_pool(name="E", bufs=2) as epool,
            tc.tile_pool(name="qk", bufs=4) as qkpool,
            tc.tile_pool(name="small", bufs=4) as spool,
            tc.tile_pool(name="y", bufs=4) as ypool,
            tc.tile_pool(name="ps", bufs=1, space="PSUM") as pspool,
        ):
            # ---- persistent SBUF tensors ----
            xT_sb = cpool.tile([128, CT, N], bf16)            # 12KB/part
            wqk_sb = cpool.tile([128, 2, CT, C], bf16)        # 18KB
            wv_sb = cpool.tile([128, CT, C], bf16)            # 9KB
            wp_sb = cpool.tile([128, CT, C], bf16)            # 9KB
            pb_sb = cpool.tile([128, CT], f32)
            eye_sb = cpool.tile([128, 128], bf16)
            v_sb = [cpool.tile([128, H, D + 1], bf16, name=f"v{nt}")
                    for nt in range(NT)]                      # 12.2KB
            on_sb = [cpool.tile([128, NCH, 512], bf16, name=f"on{kt}")
                     for kt in range(CT)]                     # 12KB
            stg_sb = [cpool.tile([128, NCH, 512], bf16, name=f"stg{otp}")
                      for otp in range(CT)]                   # 12KB

            # warmup scratch memset first so PE ramp starts ASAP
            scr = cpool.tile([128, 256], bf16, name="scr")
            nc.vector.memset(scr[:], 1.0)

            # input DMA: everything HWDGE goes on the SYNC queue (SP has no
            # compute - issuing from scalar/vector blocks that engine's SEQ
            # behind the shared HWDGE device). Ordered by first use:
            # qk0 inputs, then pair-1 weights, wv, pairs 2-5, wp. xT rows
            # 1/3 ride the gpsimd SWDGE (separate device) in parallel.
            nc.sync.dma_start(xT_sb[:, 0, 0:512], xT_d[0:128, 0:512])
            nc.sync.dma_start(xT_sb[:, 0, 512:1024], xT_d[0:128, 512:1024])
            for role in range(2):
                nc.sync.dma_start(
                    wqk_sb[:, role, 0, :],
                    wqk_d[role, 0, :, :, :].rearrange("b c d -> b (c d)"))
            nc.gpsimd.dma_start(xT_sb[:, 1, :], xT_d[128:256, :])
            nc.gpsimd.dma_start(xT_sb[:, 3, :], xT_d[384:512, :])
            nc.sync.dma_start(xT_sb[:, 2, :], xT_d[256:384, :])
            nc.sync.dma_start(xT_sb[:, 4, :], xT_d[512:640, :])
            nc.sync.dma_start(xT_sb[:, 5, :], xT_d[640:768, :])
            for role in range(2):
                nc.sync.dma_start(
                    wqk_sb[:, role, 1, :],
                    wqk_d[role, 1, :, :, :].rearrange("b c d -> b (c d)"))
            nc.sync.dma_start(wv_sb[:, :, :],
                              wv_d.rearrange("a b c -> b a c"))
            for role in range(2):
                nc.sync.dma_start(
                    wqk_sb[:, role, 2:CT, :],
                    wqk_d[role, 2:CT, :, :, :]
                    .rearrange("a b c d -> b a (c d)"))
            nc.sync.dma_start(wp_sb[:, :, :],
                              wp_d.rearrange("a b c -> b a c"))
            nc.gpsimd.dma_start(pb_sb[:], pb_d[:])
            nc.gpsimd.dma_start(eye_sb[:], eye_d[:])
            # ones column fused into v (softmax sums emerge as O col 64)
            for nt in range(NT):
                nc.vector.memset(v_sb[nt][:, :, D:D + 1], 1.0)

            def body():
                qks = {}      # (hp, role) -> SBUF tile; ("ps",hp,role) -> psum
                vps = {}
                ps_w = pspool.tile([128, 2, 512], f32, tag="f", name="warm")
                for i in range(12):
                    nc.tensor.matmul(ps_w[:, 0, 0:256], scr[:, 0:128],
                                     scr[:], start=True, stop=True)

                def emit_qk_chunk(hp, role, lo, hi, tag="f",
                                  defer_copy=False):
                    key = ("ps", hp, role)
                    if lo == 0:
                        qks[key] = pspool.tile([128, NCH, 512], f32, tag=tag,
                                               bufs=(2 if tag == "s" else 1),
                                               name=f"qk{hp}r{role}")
                    ps = qks[key]
                    for kt in range(lo, hi):
                        for nch in range(NCH):
                            nc.tensor.matmul(
                                ps[:, nch, :],
                                wqk_sb[:, role, hp, kt * 128:kt * 128 + 128],
                                xT_sb[:, kt, nch * 512:(nch + 1) * 512],
                                start=(kt == 0), stop=(kt == CT - 1),
                            )
                    if hi == CT and not defer_copy:
                        qk_copy(hp, role)

                def qk_copy(hp, role):
                    # PSUM->SBUF copy emitted at a fixed low-DVE-pressure
                    # slot so it never delays a chain-gating norm/otcopy
                    key = ("ps", hp, role)
                    ps = qks[key]
                    t = qkpool.tile([128, NCH, 512], bf16, tag="qk",
                                    name=f"qk{hp}r{role}sb")
                    if hp == 0:
                        # startup: split copies across Act (idle) and DVE
                        for nch in range(NCH):
                            if role == 0:
                                nc.scalar.copy(t[:, nch, :], ps[:, nch, :])
                            else:
                                nc.vector.tensor_copy(t[:, nch, :],
                                                      ps[:, nch, :])
                    else:
                        nc.vector.tensor_copy(t[:], ps[:, :, :])
                    qks[(hp, role)] = t
                    del qks[key]

                def emit_v_chunk(nt, lo, hi):
                    # per-och 2KB psums ride the "o" slots (idle until the
                    # phase-1 O chains, which naturally wait on v's copies)
                    if lo == 0:
                        vps[nt] = [
                            pspool.tile([128, 8, 64], f32, tag="o", bufs=2,
                                        name=f"v{nt}o{och}")
                            for och in range(2)]
                    for kt in range(lo, hi):
                        for och in range(2):
                            nc.tensor.matmul(
                                vps[nt][och][:, 0:6, :],
                                xT_sb[:, kt, nt * 128:(nt + 1) * 128],
                                wv_sb[:, kt, och * 384:(och + 1) * 384],
                                start=(kt == 0), stop=(kt == CT - 1),
                            )
                    if hi == CT:
                        for och in range(2):
                            nc.vector.tensor_copy(
                                v_sb[nt][:, och * 6:(och + 1) * 6, 0:D],
                                vps[nt][och][:, 0:6, :],
                            )
                        del vps[nt]

                fillers = []

                def take_fillers(k):
                    for _ in range(min(k, len(fillers))):
                        fillers.pop(0)()

                def queue_qk_chunks(hp):
                    for role in (0, 1):
                        for kt in range(CT):
                            fillers.append(
                                lambda hp=hp, role=role, kt=kt:
                                emit_qk_chunk(hp, role, kt, kt + 1,
                                              defer_copy=(kt == CT - 1)))

                def queue_v_chunks(nt):
                    fillers.append(lambda: emit_v_chunk(nt, 0, 3))
                    fillers.append(lambda: emit_v_chunk(nt, 3, CT))

                pj = {}

                def stage_proj(otp):
                    nc.vector.tensor_copy(stg_sb[otp][:], pj[otp][:, :, :])
                    del pj[otp]

                def proj_chunk(otp, kts, lo, stop_kt, nchs=(0, 1), tag="f"):
                    if kts[0] == lo:
                        pj[otp] = pspool.tile([128, NCH, 512], f32, tag=tag,
                                              bufs=(2 if tag == "s" else 1),
                                              name=f"pj{otp}k{lo}")
                    for kt in kts:
                        for nch in nchs:
                            nc.tensor.matmul(
                                pj[otp][:, nch, :],
                                wp_sb[:, kt, otp * 128:(otp + 1) * 128],
                                on_sb[kt][:, nch, :],
                                start=(kt == lo), stop=(kt == stop_kt),
                            )

                def queue_proj_prefill(otps, last):
                    # prefill kt 0..last (on[last] must be ready a phase
                    # before the pops land), stage partial to SBUF
                    for otp in otps:
                        for kt in range(last + 1):
                            fillers.append(
                                lambda otp=otp, kt=kt:
                                proj_chunk(otp, [kt], 0, last))
                        fillers.append(lambda otp=otp: stage_proj(otp))

                # ---- startup: eager qk(pair 0) on the S psum slots;
                # v0/v1 go through the filler queue (wv lands late).
                # A warm matmul between chunks fills each DMA-wait gap so
                # the PE p-state ramp never resets ----
                for kt in range(CT):
                    emit_qk_chunk(0, 0, kt, kt + 1, tag="s")
                    emit_qk_chunk(0, 1, kt, kt + 1, tag="s")
                    nc.tensor.matmul(ps_w[:, 1, 0:256], scr[:, 0:128],
                                     scr[:], start=True, stop=True)
                # pair-1 role-0 fills the copy window before phase 0
                for kt in range(CT):
                    emit_qk_chunk(1, 0, kt, kt + 1)

                E_prev = None

                # ---- O machinery (orientation B) ----
                def o_chain(ps_o, h, half, qi, lo=0, hi=NT, E=None):
                    """ps_o[:, qi, 0:65] += E^T @ v over key subs lo..hi-1
                    (accumulation sub-order is free: start at 0, stop at 7)."""
                    nch = half
                    qc = qi
                    ab = h % 2
                    Esrc = E_prev if E is None else E
                    for sub in range(lo, hi):
                        nc.tensor.matmul(
                            ps_o[:, qi, 0:D + 1],
                            Esrc[:, sub, nch, ab, qc * 128:(qc + 1) * 128],
                            v_sb[sub][:, h, :],
                            start=(sub == 0), stop=(sub == NT - 1),
                        )

                def o_norm(ps_o, ob, rec, act=False):
                    """rec = 1/sums, ob[q, qt, d-half] = O * rec (bf16).
                    ob is this head's 64-col half of the pair tile. act=True
                    runs the multiplies on the Activation engine."""
                    nc.vector.reciprocal_approx_fast(rec[:, :], ps_o[:, :, D])
                    for qi in range(4):
                        if act:
                            nc.scalar.activation(
                                ob[:, qi, :], ps_o[:, qi, 0:D],
                                mybir.ActivationFunctionType.Identity,
                                scale=rec[:, qi:qi + 1])
                        else:
                            nc.vector.tensor_scalar_mul(
                                ob[:, qi, :], ps_o[:, qi, 0:D],
                                rec[:, qi:qi + 1])

                def o_transpose(obp, hp0, half, act=False):
                    """ot = obp^T per qtile via eye matmul: both heads at
                    once (shared q rows) -> full-128-partition on chunk."""
                    ot = pspool.tile([128, 4, 128], f32, tag="o",
                                     bufs=2, name=f"ot{hp0}h{half}")
                    for qi in range(4):
                        nc.tensor.matmul(ot[:, qi, :], obp[:, qi, :],
                                         eye_sb[:], start=True, stop=True)
                    dst = on_sb[hp0][:, half, :]
                    if act:
                        nc.scalar.copy(dst, ot[:, :, :])
                    else:
                        nc.vector.tensor_copy(dst, ot[:, :, :])

                def new_o(h, half):
                    ps_o = pspool.tile([128, 4, 128], f32, tag="o",
                                       bufs=2, name=f"o{h}h{half}")
                    rec = spool.tile([128, 4], f32, tag="rec",
                                     name=f"rec{h}h{half}")
                    return ps_o, rec

                def new_obp(hp0, half):
                    obp = ypool.tile([128, 4, 128], bf16, tag="ob", bufs=4,
                                     name=f"obp{hp0}h{half}")
                    return obp

                for hp in range(HP):
                    q_sb, k_sb = qks[(hp, 0)], qks[(hp, 1)]
                    E_ab = epool.tile([128, NT, NCH, 2, 512], bf16, tag="E",
                                      name=f"E{hp}")
                    if hp == 0:
                        for kt in range(CT):
                            fillers.append(
                                lambda kt=kt:
                                emit_qk_chunk(1, 1, kt, kt + 1,
                                              defer_copy=(kt == CT - 1)))
                        for nt in range(NT):
                            queue_v_chunks(nt)
                    elif hp < HP - 1:
                        queue_qk_chunks(hp + 1)
                        if hp == HP - 2:
                            queue_proj_prefill((0, 1), 2)
                    else:
                        queue_proj_prefill((2, 3, 4, 5), 3)

                    ha, hb = (2 * (hp - 1), 2 * (hp - 1) + 1)
                    st = {}
                    for mt in range(NT):
                        for nch in range(NCH):
                            ps_s = pspool.tile([128, 2, 512], f32, tag="s",
                                               bufs=2, name=f"s{hp}m{mt}n{nch}")
                            c0, c1 = mt // 4, (mt % 4) * 128
                            nc.tensor.matmul(
                                ps_s[:, 0, :],
                                k_sb[0:64, c0, c1:c1 + 128],
                                q_sb[0:64, nch, :], start=True, stop=True,
                            )
                            nc.tensor.matmul(
                                ps_s[:, 1, :],
                                k_sb[64:128, c0, c1:c1 + 128],
                                q_sb[64:128, nch, :], start=True, stop=True,
                            )
                            nc.scalar.activation(
                                E_ab[:, mt, nch, :, :], ps_s[:, :, :],
                                mybir.ActivationFunctionType.Exp, scale=SCALE,
                            )
                        if mt == 2 and 0 < hp < HP - 1:
                            qk_copy(hp + 1, 0)
                        if mt == 6 and hp < HP - 1:
                            qk_copy(hp + 1, 1)
                        if E_prev is not None and hp < HP - 1:
                            # per half: a chains -> b chains -> both norms
                            # (a on DVE, b on Act) into the shared pair tile
                            # -> one paired transpose + one full-width copy
                            if mt == 0:
                                st["olo"] = new_obp(hp - 1, 0)
                                st["alo"] = new_o(ha, 0)
                                o_chain(st["alo"][0], ha, 0, 0)
                                o_chain(st["alo"][0], ha, 0, 1)
                            elif mt == 1:
                                o_chain(st["alo"][0], ha, 0, 2)
                                o_chain(st["alo"][0], ha, 0, 3)
                                o_norm(st["alo"][0], st["olo"][:, :, 0:D],
                                       st["alo"][1])
                            elif mt == 2:
                                st["blo"] = new_o(hb, 0)
                                o_chain(st["blo"][0], hb, 0, 0)
                                o_chain(st["blo"][0], hb, 0, 1)
                            elif mt == 3:
                                o_chain(st["blo"][0], hb, 0, 2)
                                o_chain(st["blo"][0], hb, 0, 3)
                                o_norm(st["blo"][0], st["olo"][:, :, D:2 * D],
                                       st["blo"][1], act=True)
                            elif mt == 4:
                                o_transpose(st["olo"], hp - 1, 0)
                                st["ohi"] = new_obp(hp - 1, 1)
                                st["ahi"] = new_o(ha, 1)
                                o_chain(st["ahi"][0], ha, 1, 0)
                                o_chain(st["ahi"][0], ha, 1, 1)
                            elif mt == 5:
                                o_chain(st["ahi"][0], ha, 1, 2)
                                o_chain(st["ahi"][0], ha, 1, 3)
                                o_norm(st["ahi"][0], st["ohi"][:, :, 0:D],
                                       st["ahi"][1])
                            elif mt == 6:
                                st["bhi"] = new_o(hb, 1)
                                o_chain(st["bhi"][0], hb, 1, 0)
                                o_chain(st["bhi"][0], hb, 1, 1)
                            elif mt == 7:
                                o_chain(st["bhi"][0], hb, 1, 2)
                                o_chain(st["bhi"][0], hb, 1, 3)
                                o_norm(st["bhi"][0], st["ohi"][:, :, D:2 * D],
                                       st["bhi"][1], act=True)
                        elif E_prev is not None:
                            # phase 5: pair-4 O compressed to mts 0-5 so the
                            # tail accumulators allocate before phase end
                            if mt == 0:
                                st["olo"] = new_obp(hp - 1, 0)
                                st["alo"] = new_o(ha, 0)
                                for qi in range(4):
                                    o_chain(st["alo"][0], ha, 0, qi)
                            elif mt == 1:
                                o_norm(st["alo"][0], st["olo"][:, :, 0:D],
                                       st["alo"][1])
                                st["blo"] = new_o(hb, 0)
                                for qi in range(4):
                                    o_chain(st["blo"][0], hb, 0, qi)
                            elif mt == 2:
                                o_norm(st["blo"][0], st["olo"][:, :, D:2 * D],
                                       st["blo"][1], act=True)
                            elif mt == 3:
                                o_transpose(st["olo"], hp - 1, 0)
                                st["ohi"] = new_obp(hp - 1, 1)
                                st["ahi"] = new_o(ha, 1)
                                for qi in range(4):
                                    o_chain(st["ahi"][0], ha, 1, qi)
                            elif mt == 4:
                                o_norm(st["ahi"][0], st["ohi"][:, :, 0:D],
                                       st["ahi"][1])
                                st["bhi"] = new_o(hb, 1)
                                for qi in range(4):
                                    o_chain(st["bhi"][0], hb, 1, qi)
                            elif mt == 5:
                                o_norm(st["bhi"][0], st["ohi"][:, :, D:2 * D],
                                       st["bhi"][1], act=True)
                            elif mt == 6:
                                o_transpose(st["ohi"], hp - 1, 1, act=True)
                                st["t10"] = new_o(H - 2, 0)
                            elif mt == 7:
                                st["t11"] = new_o(H - 1, 0)
                        nfill = 3 if (mt < 2 or len(fillers) > 8) else 2
                        take_fillers(nfill)
                    if E_prev is not None and hp < HP - 1:
                        take_fillers(2)
                        o_transpose(st["ohi"], hp - 1, 1, act=True)
                    take_fillers(len(fillers))
                    E_prev = E_ab
                    tst = st

                # ---- tail: O(pair 5) + proj kt4-5 + epilogues ----
                ha, hb = H - 2, H - 1

                yts = {}

                def proj_fin_nch(otp, nch, ap=None, dve=False):
                    # two fin flavors, alternated to balance the tail:
                    # Act(psum+bias) + DVE tt-add, or a single DVE stt
                    if otp not in yts:
                        yts[otp] = (
                            ypool.tile([128, NCH, 512], bf16, tag="y1",
                                       name=f"y1_{otp}"),
                            ypool.tile([128, NCH, 512], bf16, tag="yt",
                                       name=f"yt_{otp}"),
                        )
                    y1, yt = yts[otp]
                    src_ap = pj[otp][:, nch, :] if ap is None else ap
                    if dve:
                        nc.vector.scalar_tensor_tensor(
                            yt[:, nch, :], src_ap, pb_sb[:, otp:otp + 1],
                            stg_sb[otp][:, nch, :],
                            op0=mybir.AluOpType.add,
                            op1=mybir.AluOpType.add,
                        )
                    else:
                        nc.scalar.activation(
                            y1[:, nch, :], src_ap,
                            mybir.ActivationFunctionType.Identity,
                            bias=pb_sb[:, otp:otp + 1],
                        )
                        nc.vector.tensor_tensor(
                            yt[:, nch, :], y1[:, nch, :],
                            stg_sb[otp][:, nch, :],
                            op=mybir.AluOpType.add,
                        )
                    eng = nc.gpsimd if nch == 0 else nc.sync
                    eng.dma_start(
                        out_d[otp * 128:(otp + 1) * 128,
                              nch * 512:(nch + 1) * 512],
                        yt[:, nch, :])

                def proj_fin_staged(otp):
                    for nch in range(NCH):
                        proj_fin_nch(otp, nch)

                def proj_tail(otp, nch, tag):
                    # wave-2 proj: kt4-5 straight into a freed small slot,
                    # fin immediately (Act + DVE + DMA)
                    pjn = pspool.tile([128, 512], f32, tag=tag,
                                      bufs=(2 if tag == "o" else 1),
                                      name=f"pjt{otp}n{nch}")
                    for kt in (4, 5):
                        nc.tensor.matmul(
                            pjn[:], wp_sb[:, kt, otp * 128:(otp + 1) * 128],
                            on_sb[kt][:, nch, :],
                            start=(kt == 4), stop=(kt == 5))
                    pj[otp] = pjn
                    proj_fin_nch(otp, nch, ap=pjn[:, :], dve=True)

                # proj psums: otp0/1 -> "s" slots, otp2 -> "f"; wave-2
                # otps on the freed 2KB "o"/"f" slots. t10/t11 lo-halves were
                # pre-accumulated (subs 0-6) during phase-5 mts 6-7.
                t10, t11 = tst["t10"], tst["t11"]
                # both hi-half accumulators share one 4KB "f" tile so their
                # chains run immediately, parallel to the lo-half norm path
                obp_lo = new_obp(HP - 1, 0)
                obp_hi = new_obp(HP - 1, 1)
                thi = pspool.tile([128, 8, 128], f32, tag="f", name="thi")
                rec10h = spool.tile([128, 4], f32, tag="rec", name="rec10h")
                rec11h = spool.tile([128, 4], f32, tag="rec", name="rec11h")
                for qi in range(4):
                    o_chain(t10[0], ha, 0, qi, 0, 8, E=E_prev)
                for qi in range(4):
                    o_chain(t11[0], hb, 0, qi, 0, 8, E=E_prev)
                for qi in range(4):
                    o_chain(thi[:, 0:4, :], ha, 1, qi, E=E_prev)
                o_norm(t10[0], obp_lo[:, :, 0:D], t10[1])
                o_norm(t11[0], obp_lo[:, :, D:2 * D], t11[1], act=True)
                for qi in range(4):
                    o_chain(thi[:, 4:8, :], hb, 1, qi, E=E_prev)
                o_transpose(obp_lo, HP - 1, 0)
                proj_chunk(0, [3], 3, 5, tag="s")
                o_norm(thi[:, 0:4, :], obp_hi[:, :, 0:D], rec10h)
                o_norm(thi[:, 4:8, :], obp_hi[:, :, D:2 * D], rec11h,
                       act=True)
                proj_chunk(1, [3], 3, 5, tag="s")
                o_transpose(obp_hi, HP - 1, 1, act=True)
                proj_chunk(0, [4], 3, 5)
                proj_chunk(1, [4], 3, 5)
                proj_chunk(2, [4], 4, 5, tag="f")
                # on5 nch0 complete: kt5 nch0 for otp0-2, fin eagerly
                proj_chunk(0, [5], 3, 5, nchs=(0,))
                proj_fin_nch(0, 0)
                proj_chunk(1, [5], 3, 5, nchs=(0,))
                proj_fin_nch(1, 0)
                proj_chunk(2, [5], 4, 5, nchs=(0,))
                proj_fin_nch(2, 0)
                proj_tail(3, 0, "o")
                # wave-2: otp3 on the "o" slots, otp4/5 on the freed "s"
                # slots (allocated only after pj0/pj1 fully drain them)
                proj_chunk(0, [5], 3, 5, nchs=(1,))
                proj_fin_nch(0, 1)
                proj_chunk(4, [4], 4, 5, tag="s")
                proj_tail(3, 1, "o")
                proj_chunk(1, [5], 3, 5, nchs=(1,))
                proj_fin_nch(1, 1)
                proj_chunk(5, [4], 4, 5, tag="s")
                proj_chunk(4, [5], 4, 5, nchs=(0,))
                proj_fin_nch(4, 0)
                proj_chunk(2, [5], 4, 5, nchs=(1,))
                proj_fin_nch(2, 1)
                proj_chunk(5, [5], 4, 5, nchs=(0,))
                proj_fin_nch(5, 0)
                proj_chunk(4, [5], 4, 5, nchs=(1,))
                proj_fin_nch(4, 1)
                proj_chunk(5, [5], 4, 5, nchs=(1,))
                proj_fin_nch(5, 1, dve=True)

            if loop_r is not None:
                with tc.For_i(0, loop_r):
                    body()
            else:
                body()

    nc.compile()
    return nc


def _get_nc():
    if "nc" not in _CACHE:
        _CACHE["nc"] = _build_nc()
    return _CACHE["nc"]


def kernel(x, qkv_w, proj_w, proj_b):
    from concourse.bass_utils import run_bass_kernel_spmd

    nc = _get_nc()
    bf = ml_dtypes.bfloat16
    wqk = np.ascontiguousarray(
        qkv_w[:2 * C].T.reshape(CT, 128, 2, CT, 128)
        .transpose(2, 3, 1, 0, 4)).astype(bf)
    wv = np.ascontiguousarray(qkv_w[2 * C:].T).astype(bf).reshape(CT, 128, C)
    wp = np.ascontiguousarray(proj_w.T).astype(bf).reshape(CT, 128, C)
    pb = np.ascontiguousarray(proj_b.reshape(CT, 128).T).astype(np.float32)
    eye = np.eye(128, dtype=bf)
    in_maps = []
    for i in range(B):
        in_maps.append({
            "xT": np.ascontiguousarray(x[i].T).astype(bf),
            "wqk": wqk, "wv": wv, "wp": wp, "pb": pb, "eye": eye,
        })
    res = run_bass_kernel_spmd(nc, in_maps, core_ids=list(range(B)))
    out = np.stack([res.results[i]["out"].astype(np.float32).T for i in range(B)])
    return np.ascontiguousarray(out)


# revision 42
# speedup vs baseline: 1.0748x; 1.0748x over previous
"""Multi-head attention (B=8, N=1024, C=768, H=12) on 8 TRN2 NeuronCores.

Sharding: pure data-parallel over batch - core b computes attention for x[b].
Per-core Bass/Tile kernel, bf16 compute, f32 PSUM.

v3 schedule (orientation-B O):
  qkv/S unchanged from v2: qkv psum pairs heads on partition halves; S
  matmuls per (mt, nch) write [128 keys, 2 heads, 512 q] PSUM; one exp per
  (mt, nch) covers both heads -> E_ab[128, mt, nch, ab, 512] bf16.

  O restructured: out[q, d] = E_chunk^T @ v with E as the (free) stationary
  operand and v [128, 65] moving (ones col -> softmax sums in col 64).
  Per (head, qtile): 8 accumulating matmuls of 65 columns instead of the
  old [65, 512] orientation - halves the O column count on the PE.

  norm: DVE reciprocal of the PSUM sums column + per-partition
  tensor_scalar_mul -> normalized ob[q, d] bf16 in SBUF.

  transpose: proj needs on[d, q]; ob^T comes from a PE matmul against an
  identity matrix (ldweights are free): ot[64, 128] = ob_qt^T @ I, then one
  DVE copy per half moves [64, 4, 128] PSUM -> on_sb.

  PSUM budget (16KB/partition): tag "s" 2x[128,2,512]f32 (S double-buffer,
  also startup qk pair-0), tag "o" 2x[128,4,128]f32 (O accumulator halves
  and transpose outputs alternate through the same two slots), tag "f"
  1x4KB (warmup, qk/v/proj fillers, serial by construction).

  Per phase hp (S/exp of pair hp, O of pair hp-1), chains spread over mts:
    mt0 a-lo qt01 | mt1 a-lo qt23 +norm | mt2 b-lo qt01 | mt3 b-lo qt23
    +norm, T(a-lo)+copy | mt4 T(b-lo)+copy, a-hi qt45 | mt5 a-hi qt67
    +norm | mt6 b-hi qt45 | mt7 b-hi qt67 +norm, T(a-hi)+copy, T(b-hi)+copy
  so each "o" slot's next allocation waits only on work finished ~2 mts ago.

  tail: O(pair 5) same pattern dense; proj kt0-3 partials prestaged as
  phase-4/5 fillers (DVE stage to SBUF), tail does kt4-5 + fused epilogue
  (partial + psum + bias on DVE) + split-queue output DMA.
"""

import numpy as np
import ml_dtypes

B, N, C = 8, 1024, 768
H, D = 12, 64
SCALE = D ** -0.5
CT = C // 128        # 6 contraction tiles
NT = N // 128        # 8 token tiles
NCH = N // 512       # 2 n-chunks of 512
HP = H // 2          # 6 head pairs

_CACHE = {}


def _build_nc(loop_r=None):
    import concourse.bacc as bacc
    import concourse.mybir as mybir
    import concourse.tile as tile

    f32 = mybir.dt.float32
    bf16 = mybir.dt.bfloat16

    nc = bacc.Bacc("TRN2", target_bir_lowering=False, debug=False, num_devices=8)

    xT_d = nc.dram_tensor("xT", [C, N], bf16, kind="ExternalInput").ap()
    # weights declared row-tiled [CT, 128, ...] so one multi-descriptor DMA
    # (single HWDGE issue) can stage a whole weight with dst partitions =
    # the inner 128 rows
    # wqk layout [role, pair, row128, kt, d]: per-(role, pair-slice) DMA
    # opts to 3 dims (row, pair, kt*d) on both sides
    wqk_d = nc.dram_tensor("wqk", [2, CT, 128, CT, 128], bf16,
                           kind="ExternalInput").ap()
    wv_d = nc.dram_tensor("wv", [CT, 128, C], bf16, kind="ExternalInput").ap()
    wp_d = nc.dram_tensor("wp", [CT, 128, C], bf16, kind="ExternalInput").ap()
    pb_d = nc.dram_tensor("pb", [128, CT], f32, kind="ExternalInput").ap()
    eye_d = nc.dram_tensor("eye", [128, 128], bf16, kind="ExternalInput").ap()
    out_d = nc.dram_tensor("out", [C, N], bf16, kind="ExternalOutput").ap()

    with tile.TileContext(nc) as tc:
        with (
            tc.tile_pool(name="const", bufs=1) as cpool,
            tc.tile_pool(name="E", bufs=2) as epool,
            tc.tile_pool(name="qk", bufs=4) as qkpool,
            tc.tile_pool(name="small", bufs=4) as spool,
            tc.tile_pool(name="y", bufs=4) as ypool,
            tc.tile_pool(name="ps", bufs=1, space="PSUM") as pspool,
        ):
            # ---- persistent SBUF tensors ----
            xT_sb = cpool.tile([128, CT, N], bf16)            # 12KB/part
            wqk_sb = cpool.tile([128, 2, CT, C], bf16)        # 18KB
            wv_sb = cpool.tile([128, CT, C], bf16)            # 9KB
            wp_sb = cpool.tile([128, CT, C], bf16)            # 9KB
            pb_sb = cpool.tile([128, CT], f32)
            eye_sb = cpool.tile([128, 128], bf16)
            v_sb = [cpool.tile([128, H, D + 1], bf16, name=f"v{nt}")
                    for nt in range(NT)]                      # 12.2KB
            on_sb = [cpool.tile([128, NCH, 512], bf16, name=f"on{kt}")
                     for kt in range(CT)]                     # 12KB
            stg_sb = [cpool.tile([128, NCH, 512], bf16, name=f"stg{otp}")
                      for otp in range(CT)]                   # 12KB

            # warmup scratch memset first so PE ramp starts ASAP
            scr = cpool.tile([128, 256], bf16, name="scr")
            nc.vector.memset(scr[:], 1.0)

            # input DMA: everything HWDGE goes on the SYNC queue (SP has no
            # compute - issuing from scalar/vector blocks that engine's SEQ
            # behind the shared HWDGE device). Ordered by first use:
            # qk0 inputs, then pair-1 weights, wv, pairs 2-5, wp. xT rows
            # 1/3 ride the gpsimd SWDGE (separate device) in parallel.
            nc.sync.dma_start(xT_sb[:, 0, 0:512], xT_d[0:128, 0:512])
            nc.sync.dma_start(xT_sb[:, 0, 512:1024], xT_d[0:128, 512:1024])
            for role in range(2):
                nc.sync.dma_start(
                    wqk_sb[:, role, 0, :],
                    wqk_d[role, 0, :, :, :].rearrange("b c d -> b (c d)"))
            nc.gpsimd.dma_start(xT_sb[:, 1, :], xT_d[128:256, :])
            nc.gpsimd.dma_start(xT_sb[:, 3, :], xT_d[384:512, :])
            nc.sync.dma_start(xT_sb[:, 2, :], xT_d[256:384, :])
            nc.sync.dma_start(xT_sb[:, 4, :], xT_d[512:640, :])
            nc.sync.dma_start(xT_sb[:, 5, :], xT_d[640:768, :])
            for role in range(2):
                nc.sync.dma_start(
                    wqk_sb[:, role, 1, :],
                    wqk_d[role, 1, :, :, :].rearrange("b c d -> b (c d)"))
            nc.sync.dma_start(wv_sb[:, :, :],
                              wv_d.rearrange("a b c -> b a c"))
            for role in range(2):
                nc.sync.dma_start(
                    wqk_sb[:, role, 2:CT, :],
                    wqk_d[role, 2:CT, :, :, :]
                    .rearrange("a b c d -> b a (c d)"))
            nc.sync.dma_start(wp_sb[:, :, :],
                              wp_d.rearrange("a b c -> b a c"))
            nc.gpsimd.dma_start(pb_sb[:], pb_d[:])
            nc.gpsimd.dma_start(eye_sb[:], eye_d[:])
            # ones column fused into v (softmax sums emerge as O col 64)
            for nt in range(NT):
                nc.vector.memset(v_sb[nt][:, :, D:D + 1], 1.0)

            def body():
                qks = {}      # (hp, role) -> SBUF tile; ("ps",hp,role) -> psum
                vps = {}
                ps_w = pspool.tile([128, 2, 512], f32, tag="f", name="warm")
                for i in range(12):
                    nc.tensor.matmul(ps_w[:, 0, 0:256], scr[:, 0:128],
                                     scr[:], start=True, stop=True)

                def emit_qk_chunk(hp, role, lo, hi, tag="f",
                                  defer_copy=False):
                    key = ("ps", hp, role)
                    if lo == 0:
                        qks[key] = pspool.tile([128, NCH, 512], f32, tag=tag,
                                               bufs=(2 if tag == "s" else 1),
                                               name=f"qk{hp}r{role}")
                    ps = qks[key]
                    for kt in range(lo, hi):
                        for nch in range(NCH):
                            nc.tensor.matmul(
                                ps[:, nch, :],
                                wqk_sb[:, role, hp, kt * 128:kt * 128 + 128],
                                xT_sb[:, kt, nch * 512:(nch + 1) * 512],
                                start=(kt == 0), stop=(kt == CT - 1),
                            )
                    if hi == CT and not defer_copy:
                        qk_copy(hp, role)

                def qk_copy(hp, role):
                    # PSUM->SBUF copy emitted at a fixed low-DVE-pressure
                    # slot so it never delays a chain-gating norm/otcopy
                    key = ("ps", hp, role)
                    ps = qks[key]
                    t = qkpool.tile([128, NCH, 512], bf16, tag="qk",
                                    name=f"qk{hp}r{role}sb")
                    if hp == 0:
                        # startup: split copies across Act (idle) and DVE
                        for nch in range(NCH):
                            if role == 0:
                                nc.scalar.copy(t[:, nch, :], ps[:, nch, :])
                            else:
                                nc.vector.tensor_copy(t[:, nch, :],
                                                      ps[:, nch, :])
                    else:
                        nc.vector.tensor_copy(t[:], ps[:, :, :])
                    qks[(hp, role)] = t
                    del qks[key]

                def emit_v_chunk(nt, lo, hi):
                    # per-och 2KB psums ride the "o" slots (idle until the
                    # phase-1 O chains, which naturally wait on v's copies)
                    if lo == 0:
                        vps[nt] = [
                            pspool.tile([128, 8, 64], f32, tag="o", bufs=2,
                                        name=f"v{nt}o{och}")
                            for och in range(2)]
                    for kt in range(lo, hi):
                        for och in range(2):
                            nc.tensor.matmul(
                                vps[nt][och][:, 0:6, :],
                                xT_sb[:, kt, nt * 128:(nt + 1) * 128],
                                wv_sb[:, kt, och * 384:(och + 1) * 384],
                                start=(kt == 0), stop=(kt == CT - 1),
                            )
                    if hi == CT:
                        for och in range(2):
                            nc.vector.tensor_copy(
                                v_sb[nt][:, och * 6:(och + 1) * 6, 0:D],
                                vps[nt][och][:, 0:6, :],
                            )
                        del vps[nt]

                fillers = []

                def take_fillers(k):
                    for _ in range(min(k, len(fillers))):
                        fillers.pop(0)()

                def queue_qk_chunks(hp):
                    for role in (0, 1):
                        for kt in range(CT):
                            fillers.append(
                                lambda hp=hp, role=role, kt=kt:
                                emit_qk_chunk(hp, role, kt, kt + 1,
                                              defer_copy=(kt == CT - 1)))

                def queue_v_chunks(nt):
                    fillers.append(lambda: emit_v_chunk(nt, 0, 3))
                    fillers.append(lambda: emit_v_chunk(nt, 3, CT))

                pj = {}

                def stage_proj(otp):
                    nc.vector.tensor_copy(stg_sb[otp][:], pj[otp][:, :, :])
                    del pj[otp]

                def proj_chunk(otp, kts, lo, stop_kt, nchs=(0, 1), tag="f"):
                    if kts[0] == lo:
                        pj[otp] = pspool.tile([128, NCH, 512], f32, tag=tag,
                                              bufs=(2 if tag == "s" else 1),
                                              name=f"pj{otp}k{lo}")
                    for kt in kts:
                        for nch in nchs:
                            nc.tensor.matmul(
                                pj[otp][:, nch, :],
                                wp_sb[:, kt, otp * 128:(otp + 1) * 128],
                                on_sb[kt][:, nch, :],
                                start=(kt == lo), stop=(kt == stop_kt),
                            )

                def queue_proj_prefill(otps, last):
                    # prefill kt 0..last (on[last] must be ready a phase
                    # before the pops land), stage partial to SBUF
                    for otp in otps:
                        for kt in range(last + 1):
                            fillers.append(
                                lambda otp=otp, kt=kt:
                                proj_chunk(otp, [kt], 0, last))
                        fillers.append(lambda otp=otp: stage_proj(otp))

                # ---- startup: eager qk(pair 0) on the S psum slots;
                # v0/v1 go through the filler queue (wv lands late).
                # A warm matmul between chunks fills each DMA-wait gap so
                # the PE p-state ramp never resets ----
                for kt in range(CT):
                    emit_qk_chunk(0, 0, kt, kt + 1, tag="s")
                    emit_qk_chunk(0, 1, kt, kt + 1, tag="s")
                    nc.tensor.matmul(ps_w[:, 1, 0:256], scr[:, 0:128],
                                     scr[:], start=True, stop=True)
                # pair-1 role-0 fills the copy window before phase 0
                for kt in range(CT):
                    emit_qk_chunk(1, 0, kt, kt + 1)

                E_prev = None

                # ---- O machinery (orientation B) ----
                def o_chain(ps_o, h, half, qi, lo=0, hi=NT, E=None):
                    """ps_o[:, qi, 0:65] += E^T @ v over key subs lo..hi-1
                    (accumulation sub-order is free: start at 0, stop at 7)."""
                    nch = half
                    qc = qi
                    ab = h % 2
                    Esrc = E_prev if E is None else E
                    for sub in range(lo, hi):
                        nc.tensor.matmul(
                            ps_o[:, qi, 0:D + 1],
                            Esrc[:, sub, nch, ab, qc * 128:(qc + 1) * 128],
                            v_sb[sub][:, h, :],
                            start=(sub == 0), stop=(sub == NT - 1),
                        )

                def o_norm(ps_o, ob, rec, act=False):
                    """rec = 1/sums, ob[q, qt, d-half] = O * rec (bf16).
                    ob is this head's 64-col half of the pair tile. act=True
                    runs the multiplies on the Activation engine."""
                    nc.vector.reciprocal_approx_fast(rec[:, :], ps_o[:, :, D])
                    for qi in range(4):
                        if act:
                            nc.scalar.activation(
                                ob[:, qi, :], ps_o[:, qi, 0:D],
                                mybir.ActivationFunctionType.Identity,
                                scale=rec[:, qi:qi + 1])
                        else:
                            nc.vector.tensor_scalar_mul(
                                ob[:, qi, :], ps_o[:, qi, 0:D],
                                rec[:, qi:qi + 1])

                def o_transpose(obp, hp0, half, act=False):
                    """ot = obp^T per qtile via eye matmul: both heads at
                    once (shared q rows) -> full-128-partition on chunk."""
                    ot = pspool.tile([128, 4, 128], f32, tag="o",
                                     bufs=2, name=f"ot{hp0}h{half}")
                    for qi in range(4):
                        nc.tensor.matmul(ot[:, qi, :], obp[:, qi, :],
                                         eye_sb[:], start=True, stop=True)
                    dst = on_sb[hp0][:, half, :]
                    if act:
                        nc.scalar.copy(dst, ot[:, :, :])
                    else:
                        nc.vector.tensor_copy(dst, ot[:, :, :])

                def new_o(h, half):
                    ps_o = pspool.tile([128, 4, 128], f32, tag="o",
                                       bufs=2, name=f"o{h}h{half}")
                    rec = spool.tile([128, 4], f32, tag="rec",
                                     name=f"rec{h}h{half}")
                    return ps_o, rec

                def new_obp(hp0, half):
                    obp = ypool.tile([128, 4, 128], bf16, tag="ob", bufs=4,
                                     name=f"obp{hp0}h{half}")
                    return obp

                for hp in range(HP):
                    q_sb, k_sb = qks[(hp, 0)], qks[(hp, 1)]
                    E_ab = epool.tile([128, NT, NCH, 2, 512], bf16, tag="E",
                                      name=f"E{hp}")
                    if hp == 0:
                        for kt in range(CT):
                            fillers.append(
                                lambda kt=kt:
                                emit_qk_chunk(1, 1, kt, kt + 1,
                                              defer_copy=(kt == CT - 1)))
                        for nt in range(NT):
                            queue_v_chunks(nt)
                    elif hp < HP - 1:
                        queue_qk_chunks(hp + 1)
                        if hp == HP - 2:
                            queue_proj_prefill((0, 1), 2)
                    else:
                        queue_proj_prefill((2, 3, 4, 5), 3)

                    ha, hb = (2 * (hp - 1), 2 * (hp - 1) + 1)
                    st = {}
                    for mt in range(NT):
                        for nch in range(NCH):
                            ps_s = pspool.tile([128, 2, 512], f32, tag="s",
                                               bufs=2, name=f"s{hp}m{mt}n{nch}")
                            c0, c1 = mt // 4, (mt % 4) * 128
                            nc.tensor.matmul(
                                ps_s[:, 0, :],
                                k_sb[0:64, c0, c1:c1 + 128],
                                q_sb[0:64, nch, :], start=True, stop=True,
                            )
                            nc.tensor.matmul(
                                ps_s[:, 1, :],
                                k_sb[64:128, c0, c1:c1 + 128],
                                q_sb[64:128, nch, :], start=True, stop=True,
                            )
                            nc.scalar.activation(
                                E_ab[:, mt, nch, :, :], ps_s[:, :, :],
                                mybir.ActivationFunctionType.Exp, scale=SCALE,
                            )
                        if mt == 2 and 0 < hp < HP - 1:
                            qk_copy(hp + 1, 0)
                        if mt == 6 and hp < HP - 1:
                            qk_copy(hp + 1, 1)
                        if E_prev is not None and hp < HP - 1:
                            # per half: a chains -> b chains -> both norms
                            # (a on DVE, b on Act) into the shared pair tile
                            # -> one paired transpose + one full-width copy
                            if mt == 0:
                                st["olo"] = new_obp(hp - 1, 0)
                                st["alo"] = new_o(ha, 0)
                                o_chain(st["alo"][0], ha, 0, 0)
                                o_chain(st["alo"][0], ha, 0, 1)
                            elif mt == 1:
                                o_chain(st["alo"][0], ha, 0, 2)
                                o_chain(st["alo"][0], ha, 0, 3)
                                o_norm(st["alo"][0], st["olo"][:, :, 0:D],
                                       st["alo"][1])
                            elif mt == 2:
                                st["blo"] = new_o(hb, 0)
                                o_chain(st["blo"][0], hb, 0, 0)
                                o_chain(st["blo"][0], hb, 0, 1)
                            elif mt == 3:
                                o_chain(st["blo"][0], hb, 0, 2)
                                o_chain(st["blo"][0], hb, 0, 3)
                                o_norm(st["blo"][0], st["olo"][:, :, D:2 * D],
                                       st["blo"][1])
                            elif mt == 4:
                                o_transpose(st["olo"], hp - 1, 0)
                                st["ohi"] = new_obp(hp - 1, 1)
                                st["ahi"] = new_o(ha, 1)
                                o_chain(st["ahi"][0], ha, 1, 0)
                                o_chain(st["ahi"][0], ha, 1, 1)
                            elif mt == 5:
                                o_chain(st["ahi"][0], ha, 1, 2)
                                o_chain(st["ahi"][0], ha, 1, 3)
                                o_norm(st["ahi"][0], st["ohi"][:, :, 0:D],
                                       st["ahi"][1])
                            elif mt == 6:
                                st["bhi"] = new_o(hb, 1)
                                o_chain(st["bhi"][0], hb, 1, 0)
                                o_chain(st["bhi"][0], hb, 1, 1)
                            elif mt == 7:
                                o_chain(st["bhi"][0], hb, 1, 2)
                                o_chain(st["bhi"][0], hb, 1, 3)
                                o_norm(st["bhi"][0], st["ohi"][:, :, D:2 * D],
                                       st["bhi"][1])
                        elif E_prev is not None:
                            # phase 5: pair-4 O compressed to mts 0-5 so the
                            # tail accumulators allocate before phase end
                            if mt == 0:
                                st["olo"] = new_obp(hp - 1, 0)
                                st["alo"] = new_o(ha, 0)
                                for qi in range(4):
                                    o_chain(st["alo"][0], ha, 0, qi)
                            elif mt == 1:
                                o_norm(st["alo"][0], st["olo"][:, :, 0:D],
                                       st["alo"][1])
                                st["blo"] = new_o(hb, 0)
                                for qi in range(4):
                                    o_chain(st["blo"][0], hb, 0, qi)
                            elif mt == 2:
                                o_norm(st["blo"][0], st["olo"][:, :, D:2 * D],
                                       st["blo"][1])
                            elif mt == 3:
                                o_transpose(st["olo"], hp - 1, 0)
                                st["ohi"] = new_obp(hp - 1, 1)
                                st["ahi"] = new_o(ha, 1)
                                for qi in range(4):
                                    o_chain(st["ahi"][0], ha, 1, qi)
                            elif mt == 4:
                                o_norm(st["ahi"][0], st["ohi"][:, :, 0:D],
                                       st["ahi"][1])
                                st["bhi"] = new_o(hb, 1)
                                for qi in range(4):
                                    o_chain(st["bhi"][0], hb, 1, qi)
                            elif mt == 5:
                                o_norm(st["bhi"][0], st["ohi"][:, :, D:2 * D],
                                       st["bhi"][1])
                            elif mt == 6:
                                o_transpose(st["ohi"], hp - 1, 1)
                                st["t10"] = new_o(H - 2, 0)
                            elif mt == 7:
                                st["t11"] = new_o(H - 1, 0)
                        nfill = 3 if (mt < 2 or len(fillers) > 8) else 2
                        take_fillers(nfill)
                    if E_prev is not None and hp < HP - 1:
                        take_fillers(2)
                        o_transpose(st["ohi"], hp - 1, 1)
                    take_fillers(len(fillers))
                    E_prev = E_ab
                    tst = st

                # ---- tail: O(pair 5) + proj kt4-5 + epilogues ----
                ha, hb = H - 2, H - 1

                yts = {}

                def proj_fin_nch(otp, nch, ap=None, dve=False):
                    # two fin flavors, alternated to balance the tail:
                    # Act(psum+bias) + DVE tt-add, or a single DVE stt
                    if otp not in yts:
                        yts[otp] = (
                            ypool.tile([128, NCH, 512], bf16, tag="y1",
                                       name=f"y1_{otp}"),
                            ypool.tile([128, NCH, 512], bf16, tag="yt",
                                       name=f"yt_{otp}"),
                        )
                    y1, yt = yts[otp]
                    src_ap = pj[otp][:, nch, :] if ap is None else ap
                    if dve:
                        nc.vector.scalar_tensor_tensor(
                            yt[:, nch, :], src_ap, pb_sb[:, otp:otp + 1],
                            stg_sb[otp][:, nch, :],
                            op0=mybir.AluOpType.add,
                            op1=mybir.AluOpType.add,
                        )
                    else:
                        nc.scalar.activation(
                            y1[:, nch, :], src_ap,
                            mybir.ActivationFunctionType.Identity,
                            bias=pb_sb[:, otp:otp + 1],
                        )
                        nc.vector.tensor_tensor(
                            yt[:, nch, :], y1[:, nch, :],
                            stg_sb[otp][:, nch, :],
                            op=mybir.AluOpType.add,
                        )
                    eng = nc.gpsimd if nch == 0 else nc.sync
                    eng.dma_start(
                        out_d[otp * 128:(otp + 1) * 128,
                              nch * 512:(nch + 1) * 512],
                        yt[:, nch, :])

                def proj_fin_staged(otp):
                    for nch in range(NCH):
                        proj_fin_nch(otp, nch)

                def proj_tail(otp, nch, tag):
                    # wave-2 proj: kt4-5 straight into a freed small slot,
                    # fin immediately (Act + DVE + DMA)
                    pjn = pspool.tile([128, 512], f32, tag=tag,
                                      bufs=(2 if tag == "o" else 1),
                                      name=f"pjt{otp}n{nch}")
                    for kt in (4, 5):
                        nc.tensor.matmul(
                            pjn[:], wp_sb[:, kt, otp * 128:(otp + 1) * 128],
                            on_sb[kt][:, nch, :],
                            start=(kt == 4), stop=(kt == 5))
                    pj[otp] = pjn
                    proj_fin_nch(otp, nch, ap=pjn[:, :], dve=True)

                # proj psums: otp0/1 -> "s" slots, otp2 -> "f"; wave-2
                # otps on the freed 2KB "o"/"f" slots. t10/t11 lo-halves were
                # pre-accumulated (subs 0-6) during phase-5 mts 6-7.
                t10, t11 = tst["t10"], tst["t11"]
                # both hi-half accumulators share one 4KB "f" tile so their
                # chains run immediately, parallel to the lo-half norm path
                obp_lo = new_obp(HP - 1, 0)
                obp_hi = new_obp(HP - 1, 1)
                thi = pspool.tile([128, 8, 128], f32, tag="f", name="thi")
                rec10h = spool.tile([128, 4], f32, tag="rec", name="rec10h")
                rec11h = spool.tile([128, 4], f32, tag="rec", name="rec11h")
                for qi in range(4):
                    o_chain(t10[0], ha, 0, qi, 0, 8, E=E_prev)
                for qi in range(4):
                    o_chain(t11[0], hb, 0, qi, 0, 8, E=E_prev)
                for qi in range(4):
                    o_chain(thi[:, 0:4, :], ha, 1, qi, E=E_prev)
                o_norm(t10[0], obp_lo[:, :, 0:D], t10[1])
                o_norm(t11[0], obp_lo[:, :, D:2 * D], t11[1], act=True)
                for qi in range(4):
                    o_chain(thi[:, 4:8, :], hb, 1, qi, E=E_prev)
                o_transpose(obp_lo, HP - 1, 0)
                proj_chunk(0, [3], 3, 5, tag="s")
                o_norm(thi[:, 0:4, :], obp_hi[:, :, 0:D], rec10h)
                o_norm(thi[:, 4:8, :], obp_hi[:, :, D:2 * D], rec11h,
                       act=True)
                proj_chunk(1, [3], 3, 5, tag="s")
                o_transpose(obp_hi, HP - 1, 1, act=True)
                proj_chunk(0, [4], 3, 5)
                proj_chunk(1, [4], 3, 5)
                proj_chunk(2, [4], 4, 5, tag="f")
                # on5 nch0 complete: kt5 nch0 for otp0-2, fin eagerly
                proj_chunk(0, [5], 3, 5, nchs=(0,))
                proj_fin_nch(0, 0)
                proj_chunk(1, [5], 3, 5, nchs=(0,))
                proj_fin_nch(1, 0)
                proj_chunk(2, [5], 4, 5, nchs=(0,))
                proj_fin_nch(2, 0)
                proj_tail(3, 0, "o")
                # wave-2: otp3 on the "o" slots, otp4/5 on the freed "s"
                # slots (allocated only after pj0/pj1 fully drain them)
                proj_chunk(0, [5], 3, 5, nchs=(1,))
                proj_fin_nch(0, 1)
                proj_chunk(4, [4], 4, 5, tag="s")
                proj_tail(3, 1, "o")
                proj_chunk(1, [5], 3, 5, nchs=(1,))
                proj_fin_nch(1, 1)
                proj_chunk(5, [4], 4, 5, tag="s")
                proj_chunk(4, [5], 4, 5, nchs=(0,))
                proj_fin_nch(4, 0)
                proj_chunk(2, [5], 4, 5, nchs=(1,))
                proj_fin_nch(2, 1)
                proj_chunk(5, [5], 4, 5, nchs=(0,))
                proj_fin_nch(5, 0)
                proj_chunk(4, [5], 4, 5, nchs=(1,))
                proj_fin_nch(4, 1)
                proj_chunk(5, [5], 4, 5, nchs=(1,))
                proj_fin_nch(5, 1, dve=True)

            if loop_r is not None:
                with tc.For_i(0, loop_r):
                    body()
            else:
                body()

    nc.compile()
    return nc


def _get_nc():
    if "nc" not in _CACHE:
        _CACHE["nc"] = _build_nc()
    return _CACHE["nc"]


def kernel(x, qkv_w, proj_w, proj_b):
    from concourse.bass_utils import run_bass_kernel_spmd

    nc = _get_nc()
    bf = ml_dtypes.bfloat16
    wqk = np.ascontiguousarray(
        qkv_w[:2 * C].T.reshape(CT, 128, 2, CT, 128)
        .transpose(2, 3, 1, 0, 4)).astype(bf)
    wv = np.ascontiguousarray(qkv_w[2 * C:].T).astype(bf).reshape(CT, 128, C)
    wp = np.ascontiguousarray(proj_w.T).astype(bf).reshape(CT, 128, C)
    pb = np.ascontiguousarray(proj_b.reshape(CT, 128).T).astype(np.float32)
    eye = np.eye(128, dtype=bf)
    in_maps = []
    for i in range(B):
        in_maps.append({
            "xT": np.ascontiguousarray(x[i].T).astype(bf),
            "wqk": wqk, "wv": wv, "wp": wp, "pb": pb, "eye": eye,
        })
    res = run_bass_kernel_spmd(nc, in_maps, core_ids=list(range(B)))
    out = np.stack([res.results[i]["out"].astype(np.float32).T for i in range(B)])
    return np.ascontiguousarray(out)


# revision 44
# speedup vs baseline: 1.0844x; 1.0090x over previous
"""Multi-head attention (B=8, N=1024, C=768, H=12) on 8 TRN2 NeuronCores.

Sharding: pure data-parallel over batch - core b computes attention for x[b].
Per-core Bass/Tile kernel, bf16 compute, f32 PSUM.

v3 schedule (orientation-B O):
  qkv/S unchanged from v2: qkv psum pairs heads on partition halves; S
  matmuls per (mt, nch) write [128 keys, 2 heads, 512 q] PSUM; one exp per
  (mt, nch) covers both heads -> E_ab[128, mt, nch, ab, 512] bf16.

  O restructured: out[q, d] = E_chunk^T @ v with E as the (free) stationary
  operand and v [128, 65] moving (ones col -> softmax sums in col 64).
  Per (head, qtile): 8 accumulating matmuls of 65 columns instead of the
  old [65, 512] orientation - halves the O column count on the PE.

  norm: DVE reciprocal of the PSUM sums column + per-partition
  tensor_scalar_mul -> normalized ob[q, d] bf16 in SBUF.

  transpose: proj needs on[d, q]; ob^T comes from a PE matmul against an
  identity matrix (ldweights are free): ot[64, 128] = ob_qt^T @ I, then one
  DVE copy per half moves [64, 4, 128] PSUM -> on_sb.

  PSUM budget (16KB/partition): tag "s" 2x[128,2,512]f32 (S double-buffer,
  also startup qk pair-0), tag "o" 2x[128,4,128]f32 (O accumulator halves
  and transpose outputs alternate through the same two slots), tag "f"
  1x4KB (warmup, qk/v/proj fillers, serial by construction).

  Per phase hp (S/exp of pair hp, O of pair hp-1), chains spread over mts:
    mt0 a-lo qt01 | mt1 a-lo qt23 +norm | mt2 b-lo qt01 | mt3 b-lo qt23
    +norm, T(a-lo)+copy | mt4 T(b-lo)+copy, a-hi qt45 | mt5 a-hi qt67
    +norm | mt6 b-hi qt45 | mt7 b-hi qt67 +norm, T(a-hi)+copy, T(b-hi)+copy
  so each "o" slot's next allocation waits only on work finished ~2 mts ago.

  tail: O(pair 5) same pattern dense; proj kt0-3 partials prestaged as
  phase-4/5 fillers (DVE stage to SBUF), tail does kt4-5 + fused epilogue
  (partial + psum + bias on DVE) + split-queue output DMA.
"""

import numpy as np
import ml_dtypes

B, N, C = 8, 1024, 768
H, D = 12, 64
SCALE = D ** -0.5
CT = C // 128        # 6 contraction tiles
NT = N // 128        # 8 token tiles
NCH = N // 512       # 2 n-chunks of 512
HP = H // 2          # 6 head pairs

_CACHE = {}


def _build_nc(loop_r=None):
    import concourse.bacc as bacc
    import concourse.mybir as mybir
    import concourse.tile as tile

    f32 = mybir.dt.float32
    bf16 = mybir.dt.bfloat16

    nc = bacc.Bacc("TRN2", target_bir_lowering=False, debug=False, num_devices=8)

    xT_d = nc.dram_tensor("xT", [C, N], bf16, kind="ExternalInput").ap()
    # weights declared row-tiled [CT, 128, ...] so one multi-descriptor DMA
    # (single HWDGE issue) can stage a whole weight with dst partitions =
    # the inner 128 rows
    # wqk layout [role, pair, row128, kt, d]: per-(role, pair-slice) DMA
    # opts to 3 dims (row, pair, kt*d) on both sides
    wqk_d = nc.dram_tensor("wqk", [2, CT, 128, CT, 128], bf16,
                           kind="ExternalInput").ap()
    wv_d = nc.dram_tensor("wv", [CT, 128, C], bf16, kind="ExternalInput").ap()
    wp_d = nc.dram_tensor("wp", [CT, 128, C], bf16, kind="ExternalInput").ap()
    pb_d = nc.dram_tensor("pb", [128, CT], f32, kind="ExternalInput").ap()
    eye_d = nc.dram_tensor("eye", [128, 128], bf16, kind="ExternalInput").ap()
    out_d = nc.dram_tensor("out", [C, N], bf16, kind="ExternalOutput").ap()

    with tile.TileContext(nc) as tc:
        with (
            tc.tile_pool(name="const", bufs=1) as cpool,
            tc.tile_pool(name="E", bufs=2) as epool,
            tc.tile_pool(name="qk", bufs=4) as qkpool,
            tc.tile_pool(name="small", bufs=4) as spool,
            tc.tile_pool(name="y", bufs=4) as ypool,
            tc.tile_pool(name="ps", bufs=1, space="PSUM") as pspool,
        ):
            # ---- persistent SBUF tensors ----
            xT_sb = cpool.tile([128, CT, N], bf16)            # 12KB/part
            wqk_sb = cpool.tile([128, 2, CT, C], bf16)        # 18KB
            wv_sb = cpool.tile([128, CT, C], bf16)            # 9KB
            wp_sb = cpool.tile([128, CT, C], bf16)            # 9KB
            pb_sb = cpool.tile([128, CT], f32)
            eye_sb = cpool.tile([128, 128], bf16)
            v_sb = [cpool.tile([128, H, D + 1], bf16, name=f"v{nt}")
                    for nt in range(NT)]                      # 12.2KB
            on_sb = [cpool.tile([128, NCH, 512], bf16, name=f"on{kt}")
                     for kt in range(CT)]                     # 12KB
            stg_sb = [cpool.tile([128, NCH, 512], bf16, name=f"stg{otp}")
                      for otp in range(CT)]                   # 12KB

            # warmup scratch memset first so PE ramp starts ASAP
            scr = cpool.tile([128, 256], bf16, name="scr")
            nc.vector.memset(scr[:], 1.0)

            # input DMA: everything HWDGE goes on the SYNC queue (SP has no
            # compute - issuing from scalar/vector blocks that engine's SEQ
            # behind the shared HWDGE device). Ordered by first use:
            # qk0 inputs, then pair-1 weights, wv, pairs 2-5, wp. xT rows
            # 1/3 ride the gpsimd SWDGE (separate device) in parallel.
            nc.sync.dma_start(xT_sb[:, 0, 0:512], xT_d[0:128, 0:512])
            nc.sync.dma_start(xT_sb[:, 0, 512:1024], xT_d[0:128, 512:1024])
            for role in range(2):
                nc.sync.dma_start(
                    wqk_sb[:, role, 0, :],
                    wqk_d[role, 0, :, :, :].rearrange("b c d -> b (c d)"))
            nc.gpsimd.dma_start(xT_sb[:, 1, :], xT_d[128:256, :])
            nc.gpsimd.dma_start(xT_sb[:, 3, :], xT_d[384:512, :])
            nc.sync.dma_start(xT_sb[:, 2, :], xT_d[256:384, :])
            nc.sync.dma_start(xT_sb[:, 4, :], xT_d[512:640, :])
            nc.sync.dma_start(xT_sb[:, 5, :], xT_d[640:768, :])
            for role in range(2):
                nc.sync.dma_start(
                    wqk_sb[:, role, 1, :],
                    wqk_d[role, 1, :, :, :].rearrange("b c d -> b (c d)"))
            nc.sync.dma_start(wv_sb[:, :, :],
                              wv_d.rearrange("a b c -> b a c"))
            for role in range(2):
                nc.sync.dma_start(
                    wqk_sb[:, role, 2:CT, :],
                    wqk_d[role, 2:CT, :, :, :]
                    .rearrange("a b c d -> b a (c d)"))
            nc.sync.dma_start(wp_sb[:, :, :],
                              wp_d.rearrange("a b c -> b a c"))
            nc.gpsimd.dma_start(pb_sb[:], pb_d[:])
            nc.gpsimd.dma_start(eye_sb[:], eye_d[:])
            # ones column fused into v (softmax sums emerge as O col 64)
            for nt in range(NT):
                nc.vector.memset(v_sb[nt][:, :, D:D + 1], 1.0)

            def body():
                qks = {}      # (hp, role) -> SBUF tile; ("ps",hp,role) -> psum
                vps = {}
                ps_w = pspool.tile([128, 2, 512], f32, tag="f", name="warm")
                for i in range(8):
                    nc.tensor.matmul(ps_w[:, 0, 0:256], scr[:, 0:128],
                                     scr[:], start=True, stop=True)

                def emit_qk_chunk(hp, role, lo, hi, tag="f",
                                  defer_copy=False):
                    key = ("ps", hp, role)
                    if lo == 0:
                        qks[key] = pspool.tile([128, NCH, 512], f32, tag=tag,
                                               bufs=(2 if tag == "s" else 1),
                                               name=f"qk{hp}r{role}")
                    ps = qks[key]
                    for kt in range(lo, hi):
                        for nch in range(NCH):
                            nc.tensor.matmul(
                                ps[:, nch, :],
                                wqk_sb[:, role, hp, kt * 128:kt * 128 + 128],
                                xT_sb[:, kt, nch * 512:(nch + 1) * 512],
                                start=(kt == 0), stop=(kt == CT - 1),
                            )
                    if hi == CT and not defer_copy:
                        qk_copy(hp, role)

                def qk_copy(hp, role):
                    # PSUM->SBUF copy emitted at a fixed low-DVE-pressure
                    # slot so it never delays a chain-gating norm/otcopy
                    key = ("ps", hp, role)
                    ps = qks[key]
                    t = qkpool.tile([128, NCH, 512], bf16, tag="qk",
                                    name=f"qk{hp}r{role}sb")
                    if hp == 0:
                        # startup: split copies across Act (idle) and DVE
                        for nch in range(NCH):
                            if role == 0:
                                nc.scalar.copy(t[:, nch, :], ps[:, nch, :])
                            else:
                                nc.vector.tensor_copy(t[:, nch, :],
                                                      ps[:, nch, :])
                    else:
                        nc.vector.tensor_copy(t[:], ps[:, :, :])
                    qks[(hp, role)] = t
                    del qks[key]

                def emit_v_chunk(nt, lo, hi):
                    # per-och 2KB psums ride the "o" slots (idle until the
                    # phase-1 O chains, which naturally wait on v's copies)
                    if lo == 0:
                        vps[nt] = [
                            pspool.tile([128, 8, 64], f32, tag="o", bufs=2,
                                        name=f"v{nt}o{och}")
                            for och in range(2)]
                    for kt in range(lo, hi):
                        for och in range(2):
                            nc.tensor.matmul(
                                vps[nt][och][:, 0:6, :],
                                xT_sb[:, kt, nt * 128:(nt + 1) * 128],
                                wv_sb[:, kt, och * 384:(och + 1) * 384],
                                start=(kt == 0), stop=(kt == CT - 1),
                            )
                    if hi == CT:
                        for och in range(2):
                            nc.vector.tensor_copy(
                                v_sb[nt][:, och * 6:(och + 1) * 6, 0:D],
                                vps[nt][och][:, 0:6, :],
                            )
                        del vps[nt]

                fillers = []

                def take_fillers(k):
                    for _ in range(min(k, len(fillers))):
                        fillers.pop(0)()

                def queue_qk_chunks(hp):
                    for role in (0, 1):
                        for kt in range(CT):
                            fillers.append(
                                lambda hp=hp, role=role, kt=kt:
                                emit_qk_chunk(hp, role, kt, kt + 1,
                                              defer_copy=(kt == CT - 1)))

                def queue_v_chunks(nt):
                    fillers.append(lambda: emit_v_chunk(nt, 0, 3))
                    fillers.append(lambda: emit_v_chunk(nt, 3, CT))

                pj = {}

                def stage_proj(otp):
                    nc.vector.tensor_copy(stg_sb[otp][:], pj[otp][:, :, :])
                    del pj[otp]

                def proj_chunk(otp, kts, lo, stop_kt, nchs=(0, 1), tag="f"):
                    if kts[0] == lo:
                        pj[otp] = pspool.tile([128, NCH, 512], f32, tag=tag,
                                              bufs=(2 if tag == "s" else 1),
                                              name=f"pj{otp}k{lo}")
                    for kt in kts:
                        for nch in nchs:
                            nc.tensor.matmul(
                                pj[otp][:, nch, :],
                                wp_sb[:, kt, otp * 128:(otp + 1) * 128],
                                on_sb[kt][:, nch, :],
                                start=(kt == lo), stop=(kt == stop_kt),
                            )

                def queue_proj_prefill(otps, last):
                    # prefill kt 0..last (on[last] must be ready a phase
                    # before the pops land), stage partial to SBUF
                    for otp in otps:
                        for kt in range(last + 1):
                            fillers.append(
                                lambda otp=otp, kt=kt:
                                proj_chunk(otp, [kt], 0, last))
                        fillers.append(lambda otp=otp: stage_proj(otp))

                # ---- startup: eager qk(pair 0) on the S psum slots;
                # v0/v1 go through the filler queue (wv lands late).
                # A warm matmul between chunks fills each DMA-wait gap so
                # the PE p-state ramp never resets ----
                for kt in range(CT):
                    emit_qk_chunk(0, 0, kt, kt + 1, tag="s")
                    emit_qk_chunk(0, 1, kt, kt + 1, tag="s")
                    nc.tensor.matmul(ps_w[:, 1, 0:256], scr[:, 0:128],
                                     scr[:], start=True, stop=True)
                # pair-1 role-0 fills the copy window before phase 0
                for kt in range(CT):
                    emit_qk_chunk(1, 0, kt, kt + 1)

                E_prev = None

                # ---- O machinery (orientation B) ----
                def o_chain(ps_o, h, half, qi, lo=0, hi=NT, E=None):
                    """ps_o[:, qi, 0:65] += E^T @ v over key subs lo..hi-1
                    (accumulation sub-order is free: start at 0, stop at 7)."""
                    nch = half
                    qc = qi
                    ab = h % 2
                    Esrc = E_prev if E is None else E
                    for sub in range(lo, hi):
                        nc.tensor.matmul(
                            ps_o[:, qi, 0:D + 1],
                            Esrc[:, sub, nch, ab, qc * 128:(qc + 1) * 128],
                            v_sb[sub][:, h, :],
                            start=(sub == 0), stop=(sub == NT - 1),
                        )

                def o_norm(ps_o, ob, rec, act=False):
                    """rec = 1/sums, ob[q, qt, d-half] = O * rec (bf16).
                    ob is this head's 64-col half of the pair tile. act=True
                    runs the multiplies on the Activation engine."""
                    nc.vector.reciprocal_approx_fast(rec[:, :], ps_o[:, :, D])
                    for qi in range(4):
                        if act:
                            nc.scalar.activation(
                                ob[:, qi, :], ps_o[:, qi, 0:D],
                                mybir.ActivationFunctionType.Identity,
                                scale=rec[:, qi:qi + 1])
                        else:
                            nc.vector.tensor_scalar_mul(
                                ob[:, qi, :], ps_o[:, qi, 0:D],
                                rec[:, qi:qi + 1])

                def o_transpose(obp, hp0, half, act=False):
                    """ot = obp^T per qtile via eye matmul: both heads at
                    once (shared q rows) -> full-128-partition on chunk."""
                    ot = pspool.tile([128, 4, 128], f32, tag="o",
                                     bufs=2, name=f"ot{hp0}h{half}")
                    for qi in range(4):
                        nc.tensor.matmul(ot[:, qi, :], obp[:, qi, :],
                                         eye_sb[:], start=True, stop=True)
                    dst = on_sb[hp0][:, half, :]
                    if act:
                        nc.scalar.copy(dst, ot[:, :, :])
                    else:
                        nc.vector.tensor_copy(dst, ot[:, :, :])

                def new_o(h, half):
                    ps_o = pspool.tile([128, 4, 128], f32, tag="o",
                                       bufs=2, name=f"o{h}h{half}")
                    rec = spool.tile([128, 4], f32, tag="rec",
                                     name=f"rec{h}h{half}")
                    return ps_o, rec

                def new_obp(hp0, half):
                    obp = ypool.tile([128, 4, 128], bf16, tag="ob", bufs=4,
                                     name=f"obp{hp0}h{half}")
                    return obp

                for hp in range(HP):
                    q_sb, k_sb = qks[(hp, 0)], qks[(hp, 1)]
                    E_ab = epool.tile([128, NT, NCH, 2, 512], bf16, tag="E",
                                      name=f"E{hp}")
                    if hp == 0:
                        for kt in range(CT):
                            fillers.append(
                                lambda kt=kt:
                                emit_qk_chunk(1, 1, kt, kt + 1,
                                              defer_copy=(kt == CT - 1)))
                        for nt in range(NT):
                            queue_v_chunks(nt)
                    elif hp < HP - 1:
                        queue_qk_chunks(hp + 1)
                        if hp == HP - 2:
                            queue_proj_prefill((0, 1), 2)
                    else:
                        queue_proj_prefill((2, 3, 4, 5), 3)

                    ha, hb = (2 * (hp - 1), 2 * (hp - 1) + 1)
                    st = {}
                    for mt in range(NT):
                        for nch in range(NCH):
                            ps_s = pspool.tile([128, 2, 512], f32, tag="s",
                                               bufs=2, name=f"s{hp}m{mt}n{nch}")
                            c0, c1 = mt // 4, (mt % 4) * 128
                            nc.tensor.matmul(
                                ps_s[:, 0, :],
                                k_sb[0:64, c0, c1:c1 + 128],
                                q_sb[0:64, nch, :], start=True, stop=True,
                            )
                            nc.tensor.matmul(
                                ps_s[:, 1, :],
                                k_sb[64:128, c0, c1:c1 + 128],
                                q_sb[64:128, nch, :], start=True, stop=True,
                            )
                            nc.scalar.activation(
                                E_ab[:, mt, nch, :, :], ps_s[:, :, :],
                                mybir.ActivationFunctionType.Exp, scale=SCALE,
                            )
                        if mt == 2 and 0 < hp < HP - 1:
                            qk_copy(hp + 1, 0)
                        if mt == 6 and hp < HP - 1:
                            qk_copy(hp + 1, 1)
                        if E_prev is not None and hp < HP - 1:
                            # per half: a chains -> b chains -> both norms
                            # (a on DVE, b on Act) into the shared pair tile
                            # -> one paired transpose + one full-width copy
                            if mt == 0:
                                st["olo"] = new_obp(hp - 1, 0)
                                st["alo"] = new_o(ha, 0)
                                o_chain(st["alo"][0], ha, 0, 0)
                                o_chain(st["alo"][0], ha, 0, 1)
                            elif mt == 1:
                                o_chain(st["alo"][0], ha, 0, 2)
                                o_chain(st["alo"][0], ha, 0, 3)
                                o_norm(st["alo"][0], st["olo"][:, :, 0:D],
                                       st["alo"][1])
                            elif mt == 2:
                                st["blo"] = new_o(hb, 0)
                                o_chain(st["blo"][0], hb, 0, 0)
                                o_chain(st["blo"][0], hb, 0, 1)
                            elif mt == 3:
                                o_chain(st["blo"][0], hb, 0, 2)
                                o_chain(st["blo"][0], hb, 0, 3)
                                o_norm(st["blo"][0], st["olo"][:, :, D:2 * D],
                                       st["blo"][1])
                            elif mt == 4:
                                o_transpose(st["olo"], hp - 1, 0)
                                st["ohi"] = new_obp(hp - 1, 1)
                                st["ahi"] = new_o(ha, 1)
                                o_chain(st["ahi"][0], ha, 1, 0)
                                o_chain(st["ahi"][0], ha, 1, 1)
                            elif mt == 5:
                                o_chain(st["ahi"][0], ha, 1, 2)
                                o_chain(st["ahi"][0], ha, 1, 3)
                                o_norm(st["ahi"][0], st["ohi"][:, :, 0:D],
                                       st["ahi"][1])
                            elif mt == 6:
                                st["bhi"] = new_o(hb, 1)
                                o_chain(st["bhi"][0], hb, 1, 0)
                                o_chain(st["bhi"][0], hb, 1, 1)
                            elif mt == 7:
                                o_chain(st["bhi"][0], hb, 1, 2)
                                o_chain(st["bhi"][0], hb, 1, 3)
                                o_norm(st["bhi"][0], st["ohi"][:, :, D:2 * D],
                                       st["bhi"][1])
                        elif E_prev is not None:
                            # phase 5: pair-4 O compressed to mts 0-5 so the
                            # tail accumulators allocate before phase end
                            if mt == 0:
                                st["olo"] = new_obp(hp - 1, 0)
                                st["alo"] = new_o(ha, 0)
                                for qi in range(4):
                                    o_chain(st["alo"][0], ha, 0, qi)
                            elif mt == 1:
                                o_norm(st["alo"][0], st["olo"][:, :, 0:D],
                                       st["alo"][1])
                                st["blo"] = new_o(hb, 0)
                                for qi in range(4):
                                    o_chain(st["blo"][0], hb, 0, qi)
                            elif mt == 2:
                                o_norm(st["blo"][0], st["olo"][:, :, D:2 * D],
                                       st["blo"][1])
                            elif mt == 3:
                                o_transpose(st["olo"], hp - 1, 0)
                                st["ohi"] = new_obp(hp - 1, 1)
                                st["ahi"] = new_o(ha, 1)
                                for qi in range(4):
                                    o_chain(st["ahi"][0], ha, 1, qi)
                            elif mt == 4:
                                o_norm(st["ahi"][0], st["ohi"][:, :, 0:D],
                                       st["ahi"][1])
                                st["bhi"] = new_o(hb, 1)
                                for qi in range(4):
                                    o_chain(st["bhi"][0], hb, 1, qi)
                            elif mt == 5:
                                o_norm(st["bhi"][0], st["ohi"][:, :, D:2 * D],
                                       st["bhi"][1])
                            elif mt == 6:
                                o_transpose(st["ohi"], hp - 1, 1)
                                st["t10"] = new_o(H - 2, 0)
                            elif mt == 7:
                                st["t11"] = new_o(H - 1, 0)
                        nfill = 3 if (mt < 2 or len(fillers) > 8) else 2
                        take_fillers(nfill)
                    if E_prev is not None and hp < HP - 1:
                        take_fillers(2)
                        o_transpose(st["ohi"], hp - 1, 1)
                    take_fillers(len(fillers))
                    E_prev = E_ab
                    tst = st

                # ---- tail: O(pair 5) + proj kt4-5 + epilogues ----
                ha, hb = H - 2, H - 1

                yts = {}

                def proj_fin_nch(otp, nch, ap=None, dve=False):
                    # two fin flavors, alternated to balance the tail:
                    # Act(psum+bias) + DVE tt-add, or a single DVE stt
                    if otp not in yts:
                        yts[otp] = (
                            ypool.tile([128, NCH, 512], bf16, tag="y1",
                                       name=f"y1_{otp}"),
                            ypool.tile([128, NCH, 512], bf16, tag="yt",
                                       name=f"yt_{otp}"),
                        )
                    y1, yt = yts[otp]
                    src_ap = pj[otp][:, nch, :] if ap is None else ap
                    if dve:
                        nc.vector.scalar_tensor_tensor(
                            yt[:, nch, :], src_ap, pb_sb[:, otp:otp + 1],
                            stg_sb[otp][:, nch, :],
                            op0=mybir.AluOpType.add,
                            op1=mybir.AluOpType.add,
                        )
                    else:
                        nc.scalar.activation(
                            y1[:, nch, :], src_ap,
                            mybir.ActivationFunctionType.Identity,
                            bias=pb_sb[:, otp:otp + 1],
                        )
                        nc.vector.tensor_tensor(
                            yt[:, nch, :], y1[:, nch, :],
                            stg_sb[otp][:, nch, :],
                            op=mybir.AluOpType.add,
                        )
                    eng = nc.gpsimd if nch == 0 else nc.sync
                    eng.dma_start(
                        out_d[otp * 128:(otp + 1) * 128,
                              nch * 512:(nch + 1) * 512],
                        yt[:, nch, :])

                def proj_fin_staged(otp):
                    for nch in range(NCH):
                        proj_fin_nch(otp, nch)

                def proj_tail(otp, nch, tag):
                    # wave-2 proj: kt4-5 straight into a freed small slot,
                    # fin immediately (Act + DVE + DMA)
                    pjn = pspool.tile([128, 512], f32, tag=tag,
                                      bufs=(2 if tag == "o" else 1),
                                      name=f"pjt{otp}n{nch}")
                    for kt in (4, 5):
                        nc.tensor.matmul(
                            pjn[:], wp_sb[:, kt, otp * 128:(otp + 1) * 128],
                            on_sb[kt][:, nch, :],
                            start=(kt == 4), stop=(kt == 5))
                    pj[otp] = pjn
                    proj_fin_nch(otp, nch, ap=pjn[:, :], dve=True)

                # proj psums: otp0/1 -> "s" slots, otp2 -> "f"; wave-2
                # otps on the freed 2KB "o"/"f" slots. t10/t11 lo-halves were
                # pre-accumulated (subs 0-6) during phase-5 mts 6-7.
                t10, t11 = tst["t10"], tst["t11"]
                # both hi-half accumulators share one 4KB "f" tile so their
                # chains run immediately, parallel to the lo-half norm path
                obp_lo = new_obp(HP - 1, 0)
                obp_hi = new_obp(HP - 1, 1)
                thi = pspool.tile([128, 8, 128], f32, tag="f", name="thi")
                rec10h = spool.tile([128, 4], f32, tag="rec", name="rec10h")
                rec11h = spool.tile([128, 4], f32, tag="rec", name="rec11h")
                for qi in range(4):
                    o_chain(t10[0], ha, 0, qi, 0, 8, E=E_prev)
                for qi in range(4):
                    o_chain(t11[0], hb, 0, qi, 0, 8, E=E_prev)
                for qi in range(4):
                    o_chain(thi[:, 0:4, :], ha, 1, qi, E=E_prev)
                o_norm(t10[0], obp_lo[:, :, 0:D], t10[1])
                o_norm(t11[0], obp_lo[:, :, D:2 * D], t11[1], act=True)
                for qi in range(4):
                    o_chain(thi[:, 4:8, :], hb, 1, qi, E=E_prev)
                o_transpose(obp_lo, HP - 1, 0)
                proj_chunk(0, [3], 3, 5, tag="s")
                o_norm(thi[:, 0:4, :], obp_hi[:, :, 0:D], rec10h)
                o_norm(thi[:, 4:8, :], obp_hi[:, :, D:2 * D], rec11h,
                       act=True)
                proj_chunk(1, [3], 3, 5, tag="s")
                o_transpose(obp_hi, HP - 1, 1, act=True)
                proj_chunk(0, [4], 3, 5)
                proj_chunk(1, [4], 3, 5)
                proj_chunk(2, [4], 4, 5, tag="f")
                # on5 nch0 complete: kt5 nch0 for otp0-2, fin eagerly
                proj_chunk(0, [5], 3, 5, nchs=(0,))
                proj_fin_nch(0, 0, dve=True)
                proj_chunk(1, [5], 3, 5, nchs=(0,))
                proj_fin_nch(1, 0, dve=True)
                proj_chunk(2, [5], 4, 5, nchs=(0,))
                proj_fin_nch(2, 0, dve=True)
                proj_tail(3, 0, "o")
                # wave-2: otp3 on the "o" slots, otp4/5 on the freed "s"
                # slots (allocated only after pj0/pj1 fully drain them)
                proj_chunk(0, [5], 3, 5, nchs=(1,))
                proj_fin_nch(0, 1)
                proj_chunk(4, [4], 4, 5, tag="s")
                proj_tail(3, 1, "o")
                proj_chunk(1, [5], 3, 5, nchs=(1,))
                proj_fin_nch(1, 1, dve=True)
                proj_chunk(5, [4], 4, 5, tag="s")
                proj_chunk(4, [5], 4, 5, nchs=(0,))
                proj_fin_nch(4, 0)
                proj_chunk(2, [5], 4, 5, nchs=(1,))
                proj_fin_nch(2, 1)
                proj_chunk(5, [5], 4, 5, nchs=(0,))
                proj_fin_nch(5, 0)
                proj_chunk(4, [5], 4, 5, nchs=(1,))
                proj_fin_nch(4, 1)
                proj_chunk(5, [5], 4, 5, nchs=(1,))
                proj_fin_nch(5, 1)

            if loop_r is not None:
                with tc.For_i(0, loop_r):
                    body()
            else:
                body()

    nc.compile()
    return nc


def _get_nc():
    if "nc" not in _CACHE:
        _CACHE["nc"] = _build_nc()
    return _CACHE["nc"]


def kernel(x, qkv_w, proj_w, proj_b):
    from concourse.bass_utils import run_bass_kernel_spmd

    nc = _get_nc()
    bf = ml_dtypes.bfloat16
    wqk = np.ascontiguousarray(
        qkv_w[:2 * C].T.reshape(CT, 128, 2, CT, 128)
        .transpose(2, 3, 1, 0, 4)).astype(bf)
    wv = np.ascontiguousarray(qkv_w[2 * C:].T).astype(bf).reshape(CT, 128, C)
    wp = np.ascontiguousarray(proj_w.T).astype(bf).reshape(CT, 128, C)
    pb = np.ascontiguousarray(proj_b.reshape(CT, 128).T).astype(np.float32)
    eye = np.eye(128, dtype=bf)
    in_maps = []
    for i in range(B):
        in_maps.append({
            "xT": np.ascontiguousarray(x[i].T).astype(bf),
            "wqk": wqk, "wv": wv, "wp": wp, "pb": pb, "eye": eye,
        })
    res = run_bass_kernel_spmd(nc, in_maps, core_ids=list(range(B)))
    out = np.stack([res.results[i]["out"].astype(np.float32).T for i in range(B)])
    return np.ascontiguousarray(out)


# revision 54
# speedup vs baseline: 1.0856x; 1.0011x over previous
"""Multi-head attention (B=8, N=1024, C=768, H=12) on 8 TRN2 NeuronCores.

Sharding: pure data-parallel over batch - core b computes attention for x[b].
Per-core Bass/Tile kernel, bf16 compute, f32 PSUM.

v4 schedule (orientation-B O + paired transpose):
  qkv/S: qkv psum pairs heads on partition halves; S matmuls per (mt, nch)
  write [128 keys, 2 heads, 512 q] PSUM; one exp per (mt, nch) covers both
  heads -> E_ab[128, mt, nch, ab, 512] bf16.

  O: out[q, d] = E_chunk^T @ v with E as the (ldweights-free) stationary
  operand and v [128, 65] moving (ones col -> softmax sums in col 64).
  Per (head, qtile): 8 accumulating 65-column matmuls - half the PE column
  count of the [65, 512] orientation.

  norm: DVE reciprocal of the PSUM sums column + per-partition
  tensor_scalar_mul -> normalized ob bf16, both heads of a pair written
  into one [128, 4, 128] pair tile (head a cols 0:64, head b 64:128).

  transpose: proj needs on[d, q]; one eye-matmul per qtile transposes the
  PAIR tile (shared q rows) into a full-128-partition on chunk, then one
  copy per half moves [128, 4, 128] PSUM -> on_sb. Norm/copy engines split
  DVE/Act in the tail (Act is exp-saturated mid-phase, idle at the tail).

  PSUM (16KB/partition): tag "s" 2x[128,2,512]f32 (S double-buffer, startup
  qk pair-0, tail proj), tag "o" 2x[128,4,128]f32 (O accumulators, paired
  transposes, phase-0 v chunks), tag "f" 1x4KB (warmup, qk/proj fillers,
  tail hi-half accumulators).

  startup: all HWDGE input DMAs on the sync queue (issuing from
  scalar/vector blocks that engine's SEQ behind the shared HWDGE device),
  one merged multi-descriptor DMA per weight slice, xT split across
  HWDGE/SWDGE; warm matmuls between qk0 chunks keep the p-state ramp
  alive; qk pair-1 role-0 runs eagerly in the copy window.

  phases: per phase hp (S/exp of pair hp, O of pair hp-1) chains spread
  2/mt; phase-5 compresses pair-4 O into mts 0-5 and allocates the tail
  accumulators early. Proj kt0-2/3 partials prestaged as phase-4/5 fillers
  (DVE stage to SBUF); tail does kt3/4-5 + fused epilogue (Act psum+bias,
  DVE staged-add, or single DVE stt) + split-queue output DMA.
"""

import numpy as np
import ml_dtypes

B, N, C = 8, 1024, 768
H, D = 12, 64
SCALE = D ** -0.5
CT = C // 128        # 6 contraction tiles
NT = N // 128        # 8 token tiles
NCH = N // 512       # 2 n-chunks of 512
HP = H // 2          # 6 head pairs

_CACHE = {}


def _build_nc(loop_r=None):
    import concourse.bacc as bacc
    import concourse.mybir as mybir
    import concourse.tile as tile

    f32 = mybir.dt.float32
    bf16 = mybir.dt.bfloat16

    nc = bacc.Bacc("TRN2", target_bir_lowering=False, debug=False, num_devices=8)

    xT_d = nc.dram_tensor("xT", [C, N], bf16, kind="ExternalInput").ap()
    # weights declared row-tiled [CT, 128, ...] so one multi-descriptor DMA
    # (single HWDGE issue) can stage a whole weight with dst partitions =
    # the inner 128 rows
    # wqk layout [role, pair, row128, kt, d]: per-(role, pair-slice) DMA
    # opts to 3 dims (row, pair, kt*d) on both sides
    wqk_d = nc.dram_tensor("wqk", [2, CT, 128, CT, 128], bf16,
                           kind="ExternalInput").ap()
    wv_d = nc.dram_tensor("wv", [CT, 128, C], bf16, kind="ExternalInput").ap()
    wp_d = nc.dram_tensor("wp", [CT, 128, C], bf16, kind="ExternalInput").ap()
    pb_d = nc.dram_tensor("pb", [128, CT], f32, kind="ExternalInput").ap()
    eye_d = nc.dram_tensor("eye", [128, 128], bf16, kind="ExternalInput").ap()
    out_d = nc.dram_tensor("out", [C, N], bf16, kind="ExternalOutput").ap()

    with tile.TileContext(nc) as tc:
        with (
            tc.tile_pool(name="const", bufs=1) as cpool,
            tc.tile_pool(name="E", bufs=2) as epool,
            tc.tile_pool(name="qk", bufs=4) as qkpool,
            tc.tile_pool(name="small", bufs=4) as spool,
            tc.tile_pool(name="y", bufs=4) as ypool,
            tc.tile_pool(name="ps", bufs=1, space="PSUM") as pspool,
        ):
            # ---- persistent SBUF tensors ----
            xT_sb = cpool.tile([128, CT, N], bf16)            # 12KB/part
            wqk_sb = cpool.tile([128, 2, CT, C], bf16)        # 18KB
            wv_sb = cpool.tile([128, CT, C], bf16)            # 9KB
            wp_sb = cpool.tile([128, CT, C], bf16)            # 9KB
            pb_sb = cpool.tile([128, CT], f32)
            eye_sb = cpool.tile([128, 128], bf16)
            v_sb = [cpool.tile([128, H, D + 1], bf16, name=f"v{nt}")
                    for nt in range(NT)]                      # 12.2KB
            on_sb = [cpool.tile([128, NCH, 512], bf16, name=f"on{kt}")
                     for kt in range(CT)]                     # 12KB
            stg_sb = [cpool.tile([128, NCH, 512], bf16, name=f"stg{otp}")
                      for otp in range(CT)]                   # 12KB

            # warmup scratch memset first so PE ramp starts ASAP
            scr = cpool.tile([128, 256], bf16, name="scr")
            nc.vector.memset(scr[:], 1.0)

            # input DMA: everything HWDGE goes on the SYNC queue (SP has no
            # compute - issuing from scalar/vector blocks that engine's SEQ
            # behind the shared HWDGE device). Ordered by first use:
            # qk0 inputs, then pair-1 weights, wv, pairs 2-5, wp. xT rows
            # 1/3 ride the gpsimd SWDGE (separate device) in parallel.
            nc.sync.dma_start(xT_sb[:, 0, 0:512], xT_d[0:128, 0:512])
            nc.sync.dma_start(xT_sb[:, 0, 512:1024], xT_d[0:128, 512:1024])
            for role in range(2):
                nc.sync.dma_start(
                    wqk_sb[:, role, 0, :],
                    wqk_d[role, 0, :, :, :].rearrange("b c d -> b (c d)"))
            nc.gpsimd.dma_start(xT_sb[:, 1, :], xT_d[128:256, :])
            nc.gpsimd.dma_start(xT_sb[:, 3, :], xT_d[384:512, :])
            nc.sync.dma_start(xT_sb[:, 2, :], xT_d[256:384, :])
            nc.sync.dma_start(xT_sb[:, 4, :], xT_d[512:640, :])
            nc.sync.dma_start(xT_sb[:, 5, :], xT_d[640:768, :])
            for role in range(2):
                nc.sync.dma_start(
                    wqk_sb[:, role, 1, :],
                    wqk_d[role, 1, :, :, :].rearrange("b c d -> b (c d)"))
            nc.sync.dma_start(wv_sb[:, :, :],
                              wv_d.rearrange("a b c -> b a c"))
            for role in range(2):
                nc.sync.dma_start(
                    wqk_sb[:, role, 2:CT, :],
                    wqk_d[role, 2:CT, :, :, :]
                    .rearrange("a b c d -> b a (c d)"))
            nc.sync.dma_start(wp_sb[:, :, :],
                              wp_d.rearrange("a b c -> b a c"))
            nc.gpsimd.dma_start(pb_sb[:], pb_d[:])
            nc.gpsimd.dma_start(eye_sb[:], eye_d[:])
            # ones column fused into v (softmax sums emerge as O col 64)
            for nt in range(NT):
                nc.vector.memset(v_sb[nt][:, :, D:D + 1], 1.0)

            def body():
                qks = {}      # (hp, role) -> SBUF tile; ("ps",hp,role) -> psum
                vps = {}
                ps_w = pspool.tile([128, 2, 512], f32, tag="f", name="warm")
                for i in range(8):
                    nc.tensor.matmul(ps_w[:, 0, 0:256], scr[:, 0:128],
                                     scr[:], start=True, stop=True)

                def emit_qk_chunk(hp, role, lo, hi, tag="f",
                                  defer_copy=False):
                    key = ("ps", hp, role)
                    if lo == 0:
                        qks[key] = pspool.tile([128, NCH, 512], f32, tag=tag,
                                               bufs=(2 if tag == "s" else 1),
                                               name=f"qk{hp}r{role}")
                    ps = qks[key]
                    for kt in range(lo, hi):
                        for nch in range(NCH):
                            nc.tensor.matmul(
                                ps[:, nch, :],
                                wqk_sb[:, role, hp, kt * 128:kt * 128 + 128],
                                xT_sb[:, kt, nch * 512:(nch + 1) * 512],
                                start=(kt == 0), stop=(kt == CT - 1),
                            )
                    if hi == CT and not defer_copy:
                        qk_copy(hp, role)

                def qk_copy(hp, role):
                    # PSUM->SBUF copy emitted at a fixed low-DVE-pressure
                    # slot so it never delays a chain-gating norm/otcopy
                    key = ("ps", hp, role)
                    ps = qks[key]
                    t = qkpool.tile([128, NCH, 512], bf16, tag="qk",
                                    name=f"qk{hp}r{role}sb")
                    if hp == 0:
                        # startup: split copies across Act (idle) and DVE
                        for nch in range(NCH):
                            if role == 0:
                                nc.scalar.copy(t[:, nch, :], ps[:, nch, :])
                            else:
                                nc.vector.tensor_copy(t[:, nch, :],
                                                      ps[:, nch, :])
                    else:
                        nc.vector.tensor_copy(t[:], ps[:, :, :])
                    qks[(hp, role)] = t
                    del qks[key]

                def emit_v_chunk(nt, lo, hi):
                    # per-och 2KB psums ride the "o" slots (idle until the
                    # phase-1 O chains, which naturally wait on v's copies)
                    if lo == 0:
                        vps[nt] = [
                            pspool.tile([128, 8, 64], f32, tag="o", bufs=2,
                                        name=f"v{nt}o{och}")
                            for och in range(2)]
                    for kt in range(lo, hi):
                        for och in range(2):
                            nc.tensor.matmul(
                                vps[nt][och][:, 0:6, :],
                                xT_sb[:, kt, nt * 128:(nt + 1) * 128],
                                wv_sb[:, kt, och * 384:(och + 1) * 384],
                                start=(kt == 0), stop=(kt == CT - 1),
                            )
                    if hi == CT:
                        for och in range(2):
                            nc.vector.tensor_copy(
                                v_sb[nt][:, och * 6:(och + 1) * 6, 0:D],
                                vps[nt][och][:, 0:6, :],
                            )
                        del vps[nt]

                fillers = []

                def take_fillers(k):
                    for _ in range(min(k, len(fillers))):
                        fillers.pop(0)()

                def queue_qk_chunks(hp):
                    for role in (0, 1):
                        for kt in range(CT):
                            fillers.append(
                                lambda hp=hp, role=role, kt=kt:
                                emit_qk_chunk(hp, role, kt, kt + 1,
                                              defer_copy=(kt == CT - 1)))

                def queue_v_chunks(nt):
                    fillers.append(lambda: emit_v_chunk(nt, 0, 3))
                    fillers.append(lambda: emit_v_chunk(nt, 3, CT))

                pj = {}

                def stage_proj(otp):
                    nc.vector.tensor_copy(stg_sb[otp][:], pj[otp][:, :, :])
                    del pj[otp]

                def proj_chunk(otp, kts, lo, stop_kt, nchs=(0, 1), tag="f"):
                    if kts[0] == lo:
                        pj[otp] = pspool.tile([128, NCH, 512], f32, tag=tag,
                                              bufs=(2 if tag == "s" else 1),
                                              name=f"pj{otp}k{lo}")
                    for kt in kts:
                        for nch in nchs:
                            nc.tensor.matmul(
                                pj[otp][:, nch, :],
                                wp_sb[:, kt, otp * 128:(otp + 1) * 128],
                                on_sb[kt][:, nch, :],
                                start=(kt == lo), stop=(kt == stop_kt),
                            )

                def queue_proj_prefill(otps, last):
                    # prefill kt 0..last (on[last] must be ready a phase
                    # before the pops land), stage partial to SBUF
                    for otp in otps:
                        for kt in range(last + 1):
                            fillers.append(
                                lambda otp=otp, kt=kt:
                                proj_chunk(otp, [kt], 0, last))
                        fillers.append(lambda otp=otp: stage_proj(otp))

                # ---- startup: eager qk(pair 0) on the S psum slots;
                # v0/v1 go through the filler queue (wv lands late).
                # A warm matmul between chunks fills each DMA-wait gap so
                # the PE p-state ramp never resets ----
                for kt in range(CT):
                    emit_qk_chunk(0, 0, kt, kt + 1, tag="s")
                    emit_qk_chunk(0, 1, kt, kt + 1, tag="s")
                    nc.tensor.matmul(ps_w[:, 1, 0:256], scr[:, 0:128],
                                     scr[:], start=True, stop=True)
                # pair-1 role-0 fills the copy window before phase 0
                for kt in range(CT):
                    emit_qk_chunk(1, 0, kt, kt + 1)

                E_prev = None

                # ---- O machinery (orientation B) ----
                def o_chain(ps_o, h, half, qi, lo=0, hi=NT, E=None):
                    """ps_o[:, qi, 0:65] += E^T @ v over key subs lo..hi-1
                    (accumulation sub-order is free: start at 0, stop at 7)."""
                    nch = half
                    qc = qi
                    ab = h % 2
                    Esrc = E_prev if E is None else E
                    for sub in range(lo, hi):
                        nc.tensor.matmul(
                            ps_o[:, qi, 0:D + 1],
                            Esrc[:, sub, nch, ab, qc * 128:(qc + 1) * 128],
                            v_sb[sub][:, h, :],
                            start=(sub == 0), stop=(sub == NT - 1),
                        )

                def o_norm(ps_o, ob, rec, act=False):
                    """rec = 1/sums, ob[q, qt, d-half] = O * rec (bf16).
                    ob is this head's 64-col half of the pair tile. act=True
                    runs the multiplies on the Activation engine."""
                    nc.vector.reciprocal_approx_fast(rec[:, :], ps_o[:, :, D])
                    for qi in range(4):
                        if act:
                            nc.scalar.activation(
                                ob[:, qi, :], ps_o[:, qi, 0:D],
                                mybir.ActivationFunctionType.Identity,
                                scale=rec[:, qi:qi + 1])
                        else:
                            nc.vector.tensor_scalar_mul(
                                ob[:, qi, :], ps_o[:, qi, 0:D],
                                rec[:, qi:qi + 1])

                def o_transpose(obp, hp0, half, act=False):
                    """ot = obp^T per qtile via eye matmul: both heads at
                    once (shared q rows) -> full-128-partition on chunk."""
                    ot = pspool.tile([128, 4, 128], f32, tag="o",
                                     bufs=2, name=f"ot{hp0}h{half}")
                    for qi in range(4):
                        nc.tensor.matmul(ot[:, qi, :], obp[:, qi, :],
                                         eye_sb[:], start=True, stop=True)
                    dst = on_sb[hp0][:, half, :]
                    if act:
                        nc.scalar.copy(dst, ot[:, :, :])
                    else:
                        nc.vector.tensor_copy(dst, ot[:, :, :])

                def new_o(h, half):
                    ps_o = pspool.tile([128, 4, 128], f32, tag="o",
                                       bufs=2, name=f"o{h}h{half}")
                    rec = spool.tile([128, 4], f32, tag="rec",
                                     name=f"rec{h}h{half}")
                    return ps_o, rec

                def new_obp(hp0, half):
                    obp = ypool.tile([128, 4, 128], bf16, tag="ob", bufs=4,
                                     name=f"obp{hp0}h{half}")
                    return obp

                for hp in range(HP):
                    q_sb, k_sb = qks[(hp, 0)], qks[(hp, 1)]
                    E_ab = epool.tile([128, NT, NCH, 2, 512], bf16, tag="E",
                                      name=f"E{hp}")
                    if hp == 0:
                        for kt in range(CT):
                            fillers.append(
                                lambda kt=kt:
                                emit_qk_chunk(1, 1, kt, kt + 1,
                                              defer_copy=(kt == CT - 1)))
                        for nt in range(NT):
                            queue_v_chunks(nt)
                    elif hp < HP - 1:
                        queue_qk_chunks(hp + 1)
                        if hp == HP - 2:
                            queue_proj_prefill((0, 1), 2)
                    else:
                        queue_proj_prefill((2, 3, 4, 5), 3)

                    ha, hb = (2 * (hp - 1), 2 * (hp - 1) + 1)
                    st = {}
                    for mt in range(NT):
                        for nch in range(NCH):
                            ps_s = pspool.tile([128, 2, 512], f32, tag="s",
                                               bufs=2, name=f"s{hp}m{mt}n{nch}")
                            c0, c1 = mt // 4, (mt % 4) * 128
                            nc.tensor.matmul(
                                ps_s[:, 0, :],
                                k_sb[0:64, c0, c1:c1 + 128],
                                q_sb[0:64, nch, :], start=True, stop=True,
                            )
                            nc.tensor.matmul(
                                ps_s[:, 1, :],
                                k_sb[64:128, c0, c1:c1 + 128],
                                q_sb[64:128, nch, :], start=True, stop=True,
                            )
                            nc.scalar.activation(
                                E_ab[:, mt, nch, :, :], ps_s[:, :, :],
                                mybir.ActivationFunctionType.Exp, scale=SCALE,
                            )
                        if mt == 2 and 0 < hp < HP - 1:
                            qk_copy(hp + 1, 0)
                        if mt == 6 and hp < HP - 1:
                            qk_copy(hp + 1, 1)
                        if E_prev is not None and hp < HP - 1:
                            # per half: a chains -> b chains -> both norms
                            # (a on DVE, b on Act) into the shared pair tile
                            # -> one paired transpose + one full-width copy
                            if mt == 0:
                                st["olo"] = new_obp(hp - 1, 0)
                                st["alo"] = new_o(ha, 0)
                                o_chain(st["alo"][0], ha, 0, 0)
                                o_chain(st["alo"][0], ha, 0, 1)
                            elif mt == 1:
                                o_chain(st["alo"][0], ha, 0, 2)
                                o_chain(st["alo"][0], ha, 0, 3)
                                o_norm(st["alo"][0], st["olo"][:, :, 0:D],
                                       st["alo"][1])
                            elif mt == 2:
                                st["blo"] = new_o(hb, 0)
                                o_chain(st["blo"][0], hb, 0, 0)
                                o_chain(st["blo"][0], hb, 0, 1)
                            elif mt == 3:
                                o_chain(st["blo"][0], hb, 0, 2)
                                o_chain(st["blo"][0], hb, 0, 3)
                                o_norm(st["blo"][0], st["olo"][:, :, D:2 * D],
                                       st["blo"][1])
                            elif mt == 4:
                                o_transpose(st["olo"], hp - 1, 0)
                                st["ohi"] = new_obp(hp - 1, 1)
                                st["ahi"] = new_o(ha, 1)
                                o_chain(st["ahi"][0], ha, 1, 0)
                                o_chain(st["ahi"][0], ha, 1, 1)
                            elif mt == 5:
                                o_chain(st["ahi"][0], ha, 1, 2)
                                o_chain(st["ahi"][0], ha, 1, 3)
                                o_norm(st["ahi"][0], st["ohi"][:, :, 0:D],
                                       st["ahi"][1])
                            elif mt == 6:
                                st["bhi"] = new_o(hb, 1)
                                o_chain(st["bhi"][0], hb, 1, 0)
                                o_chain(st["bhi"][0], hb, 1, 1)
                            elif mt == 7:
                                o_chain(st["bhi"][0], hb, 1, 2)
                                o_chain(st["bhi"][0], hb, 1, 3)
                                o_norm(st["bhi"][0], st["ohi"][:, :, D:2 * D],
                                       st["bhi"][1])
                        elif E_prev is not None:
                            # phase 5: pair-4 O compressed to mts 0-5 so the
                            # tail accumulators allocate before phase end
                            if mt == 0:
                                st["olo"] = new_obp(hp - 1, 0)
                                st["alo"] = new_o(ha, 0)
                                for qi in range(4):
                                    o_chain(st["alo"][0], ha, 0, qi)
                            elif mt == 1:
                                o_norm(st["alo"][0], st["olo"][:, :, 0:D],
                                       st["alo"][1])
                                st["blo"] = new_o(hb, 0)
                                for qi in range(4):
                                    o_chain(st["blo"][0], hb, 0, qi)
                            elif mt == 2:
                                o_norm(st["blo"][0], st["olo"][:, :, D:2 * D],
                                       st["blo"][1])
                            elif mt == 3:
                                o_transpose(st["olo"], hp - 1, 0)
                                st["ohi"] = new_obp(hp - 1, 1)
                                st["ahi"] = new_o(ha, 1)
                                for qi in range(4):
                                    o_chain(st["ahi"][0], ha, 1, qi)
                            elif mt == 4:
                                o_norm(st["ahi"][0], st["ohi"][:, :, 0:D],
                                       st["ahi"][1])
                                st["bhi"] = new_o(hb, 1)
                                for qi in range(4):
                                    o_chain(st["bhi"][0], hb, 1, qi)
                            elif mt == 5:
                                o_norm(st["bhi"][0], st["ohi"][:, :, D:2 * D],
                                       st["bhi"][1])
                            elif mt == 6:
                                o_transpose(st["ohi"], hp - 1, 1)
                                st["t10"] = new_o(H - 2, 0)
                            elif mt == 7:
                                st["t11"] = new_o(H - 1, 0)
                        nfill = 3
                        take_fillers(nfill)
                    if E_prev is not None and hp < HP - 1:
                        take_fillers(2)
                        o_transpose(st["ohi"], hp - 1, 1)
                    take_fillers(len(fillers))
                    E_prev = E_ab
                    tst = st

                # ---- tail: O(pair 5) + proj kt4-5 + epilogues ----
                ha, hb = H - 2, H - 1

                yts = {}

                def proj_fin_nch(otp, nch, ap=None, dve=False):
                    # two fin flavors, alternated to balance the tail:
                    # Act(psum+bias) + DVE tt-add, or a single DVE stt
                    if otp not in yts:
                        yts[otp] = (
                            ypool.tile([128, NCH, 512], bf16, tag="y1",
                                       name=f"y1_{otp}"),
                            ypool.tile([128, NCH, 512], bf16, tag="yt",
                                       name=f"yt_{otp}"),
                        )
                    y1, yt = yts[otp]
                    src_ap = pj[otp][:, nch, :] if ap is None else ap
                    if dve:
                        nc.vector.scalar_tensor_tensor(
                            yt[:, nch, :], src_ap, pb_sb[:, otp:otp + 1],
                            stg_sb[otp][:, nch, :],
                            op0=mybir.AluOpType.add,
                            op1=mybir.AluOpType.add,
                        )
                    else:
                        nc.scalar.activation(
                            y1[:, nch, :], src_ap,
                            mybir.ActivationFunctionType.Identity,
                            bias=pb_sb[:, otp:otp + 1],
                        )
                        nc.vector.tensor_tensor(
                            yt[:, nch, :], y1[:, nch, :],
                            stg_sb[otp][:, nch, :],
                            op=mybir.AluOpType.add,
                        )
                    eng = nc.gpsimd if nch == 0 else nc.sync
                    eng.dma_start(
                        out_d[otp * 128:(otp + 1) * 128,
                              nch * 512:(nch + 1) * 512],
                        yt[:, nch, :])

                def proj_fin_staged(otp):
                    for nch in range(NCH):
                        proj_fin_nch(otp, nch)

                def proj_tail(otp, nch, tag):
                    # wave-2 proj: kt4-5 straight into a freed small slot,
                    # fin immediately (Act + DVE + DMA)
                    pjn = pspool.tile([128, 512], f32, tag=tag,
                                      bufs=(2 if tag == "o" else 1),
                                      name=f"pjt{otp}n{nch}")
                    for kt in (4, 5):
                        nc.tensor.matmul(
                            pjn[:], wp_sb[:, kt, otp * 128:(otp + 1) * 128],
                            on_sb[kt][:, nch, :],
                            start=(kt == 4), stop=(kt == 5))
                    pj[otp] = pjn
                    proj_fin_nch(otp, nch, ap=pjn[:, :], dve=True)

                # proj psums: otp0/1 -> "s" slots, otp2 -> "f"; wave-2
                # otps on the freed 2KB "o"/"f" slots. t10/t11 lo-halves were
                # pre-accumulated (subs 0-6) during phase-5 mts 6-7.
                t10, t11 = tst["t10"], tst["t11"]
                # both hi-half accumulators share one 4KB "f" tile so their
                # chains run immediately, parallel to the lo-half norm path
                obp_lo = new_obp(HP - 1, 0)
                obp_hi = new_obp(HP - 1, 1)
                thi = pspool.tile([128, 8, 128], f32, tag="f", name="thi")
                rec10h = spool.tile([128, 4], f32, tag="rec", name="rec10h")
                rec11h = spool.tile([128, 4], f32, tag="rec", name="rec11h")
                for qi in range(4):
                    o_chain(t10[0], ha, 0, qi, 0, 8, E=E_prev)
                for qi in range(4):
                    o_chain(t11[0], hb, 0, qi, 0, 8, E=E_prev)
                for qi in range(4):
                    o_chain(thi[:, 0:4, :], ha, 1, qi, E=E_prev)
                o_norm(t10[0], obp_lo[:, :, 0:D], t10[1])
                o_norm(t11[0], obp_lo[:, :, D:2 * D], t11[1], act=True)
                for qi in range(4):
                    o_chain(thi[:, 4:8, :], hb, 1, qi, E=E_prev)
                o_transpose(obp_lo, HP - 1, 0)
                proj_chunk(0, [3], 3, 5, tag="s")
                o_norm(thi[:, 0:4, :], obp_hi[:, :, 0:D], rec10h)
                o_norm(thi[:, 4:8, :], obp_hi[:, :, D:2 * D], rec11h,
                       act=True)
                proj_chunk(1, [3], 3, 5, tag="s")
                o_transpose(obp_hi, HP - 1, 1, act=True)
                proj_chunk(0, [4], 3, 5)
                proj_chunk(1, [4], 3, 5)
                proj_chunk(2, [4], 4, 5, tag="f")
                # on5 nch0 complete: kt5 nch0 for otp0-2, fin eagerly
                proj_chunk(0, [5], 3, 5, nchs=(0,))
                proj_fin_nch(0, 0, dve=True)
                proj_chunk(1, [5], 3, 5, nchs=(0,))
                proj_fin_nch(1, 0, dve=True)
                proj_chunk(2, [5], 4, 5, nchs=(0,))
                proj_fin_nch(2, 0, dve=True)
                proj_tail(3, 0, "o")
                # wave-2: otp3 on the "o" slots, otp4/5 on the freed "s"
                # slots (allocated only after pj0/pj1 fully drain them)
                proj_chunk(0, [5], 3, 5, nchs=(1,))
                proj_fin_nch(0, 1)
                proj_chunk(4, [4], 4, 5, tag="s")
                proj_tail(3, 1, "o")
                proj_chunk(1, [5], 3, 5, nchs=(1,))
                proj_fin_nch(1, 1, dve=True)
                proj_chunk(5, [4], 4, 5, tag="s")
                proj_chunk(4, [5], 4, 5, nchs=(0,))
                proj_fin_nch(4, 0)
                proj_chunk(2, [5], 4, 5, nchs=(1,))
                proj_fin_nch(2, 1)
                proj_chunk(5, [5], 4, 5, nchs=(0,))
                proj_fin_nch(5, 0)
                proj_chunk(4, [5], 4, 5, nchs=(1,))
                proj_fin_nch(4, 1)
                proj_chunk(5, [5], 4, 5, nchs=(1,))
                proj_fin_nch(5, 1)

            if loop_r is not None:
                with tc.For_i(0, loop_r):
                    body()
            else:
                body()

    nc.compile()
    return nc


def _get_nc():
    if "nc" not in _CACHE:
        _CACHE["nc"] = _build_nc()
    return _CACHE["nc"]


def kernel(x, qkv_w, proj_w, proj_b):
    from concourse.bass_utils import run_bass_kernel_spmd

    nc = _get_nc()
    bf = ml_dtypes.bfloat16
    wqk = np.ascontiguousarray(
        qkv_w[:2 * C].T.reshape(CT, 128, 2, CT, 128)
        .transpose(2, 3, 1, 0, 4)).astype(bf)
    wv = np.ascontiguousarray(qkv_w[2 * C:].T).astype(bf).reshape(CT, 128, C)
    wp = np.ascontiguousarray(proj_w.T).astype(bf).reshape(CT, 128, C)
    pb = np.ascontiguousarray(proj_b.reshape(CT, 128).T).astype(np.float32)
    eye = np.eye(128, dtype=bf)
    in_maps = []
    for i in range(B):
        in_maps.append({
            "xT": np.ascontiguousarray(x[i].T).astype(bf),
            "wqk": wqk, "wv": wv, "wp": wp, "pb": pb, "eye": eye,
        })
    res = run_bass_kernel_spmd(nc, in_maps, core_ids=list(range(B)))
    out = np.stack([res.results[i]["out"].astype(np.float32).T for i in range(B)])
    return np.ascontiguousarray(out)


# revision 63
# speedup vs baseline: 1.1164x; 1.0284x over previous
"""Multi-head attention (B=8, N=1024, C=768, H=12) on 8 TRN2 NeuronCores.

Sharding: pure data-parallel over batch - core b computes attention for x[b].
Per-core Bass/Tile kernel, bf16 compute, f32 PSUM.

v4 schedule (orientation-B O + paired transpose):
  qkv/S: qkv psum pairs heads on partition halves; S matmuls per (mt, nch)
  write [128 keys, 2 heads, 512 q] PSUM; one exp per (mt, nch) covers both
  heads -> E_ab[128, mt, nch, ab, 512] bf16.

  O: out[q, d] = E_chunk^T @ v with E as the (ldweights-free) stationary
  operand and v [128, 65] moving (ones col -> softmax sums in col 64).
  Per (head, qtile): 8 accumulating 65-column matmuls - half the PE column
  count of the [65, 512] orientation.

  norm: DVE reciprocal of the PSUM sums column + ONE broadcast-AP
  (stride-0) tensor_tensor multiply per head covering all 4 qtiles ->
  normalized ob bf16, both heads of a pair written into one [128, 4, 128]
  pair tile (head a cols 0:64, head b 64:128).

  transpose: proj needs on[d, q]; one eye-matmul per qtile transposes the
  PAIR tile (shared q rows) into a full-128-partition on chunk, then one
  copy per half moves [128, 4, 128] PSUM -> on_sb. Norm/copy engines split
  DVE/Act in the tail (Act is exp-saturated mid-phase, idle at the tail).

  PSUM (16KB/partition): tag "s" 2x[128,2,512]f32 (S double-buffer, startup
  qk pair-0, tail proj), tag "o" 2x[128,4,128]f32 (O accumulators, paired
  transposes, phase-0 v chunks), tag "f" 1x4KB (warmup, qk/proj fillers,
  tail hi-half accumulators).

  startup: all HWDGE input DMAs on the sync queue (issuing from
  scalar/vector blocks that engine's SEQ behind the shared HWDGE device),
  one merged multi-descriptor DMA per weight slice, xT split across
  HWDGE/SWDGE; warm matmuls between qk0 chunks keep the p-state ramp
  alive; qk pair-1 role-0 runs eagerly in the copy window.

  phases: per phase hp (S/exp of pair hp, O of pair hp-1) chains spread
  2/mt; phase-5 compresses pair-4 O into mts 0-5 and allocates the tail
  accumulators early. Proj kt0-2/3 partials prestaged as phase-4/5 fillers
  (DVE stage to SBUF); tail does kt3/4-5 + fused epilogue (Act psum+bias,
  DVE staged-add, or single DVE stt) + split-queue output DMA.
"""

import numpy as np
import ml_dtypes

B, N, C = 8, 1024, 768
H, D = 12, 64
SCALE = D ** -0.5
CT = C // 128        # 6 contraction tiles
NT = N // 128        # 8 token tiles
NCH = N // 512       # 2 n-chunks of 512
HP = H // 2          # 6 head pairs

_CACHE = {}


def _build_nc(loop_r=None):
    import concourse.bacc as bacc
    import concourse.mybir as mybir
    import concourse.tile as tile

    f32 = mybir.dt.float32
    bf16 = mybir.dt.bfloat16

    nc = bacc.Bacc("TRN2", target_bir_lowering=False, debug=False, num_devices=8)

    xT_d = nc.dram_tensor("xT", [C, N], bf16, kind="ExternalInput").ap()
    # weights declared row-tiled [CT, 128, ...] so one multi-descriptor DMA
    # (single HWDGE issue) can stage a whole weight with dst partitions =
    # the inner 128 rows
    # wqk layout [role, pair, row128, kt, d]: per-(role, pair-slice) DMA
    # opts to 3 dims (row, pair, kt*d) on both sides
    wqk_d = nc.dram_tensor("wqk", [2, CT, 128, CT, 128], bf16,
                           kind="ExternalInput").ap()
    wv_d = nc.dram_tensor("wv", [CT, 128, C], bf16, kind="ExternalInput").ap()
    wp_d = nc.dram_tensor("wp", [CT, 128, C], bf16, kind="ExternalInput").ap()
    pb_d = nc.dram_tensor("pb", [128, CT], f32, kind="ExternalInput").ap()
    eye_d = nc.dram_tensor("eye", [128, 128], bf16, kind="ExternalInput").ap()
    out_d = nc.dram_tensor("out", [C, N], bf16, kind="ExternalOutput").ap()

    with tile.TileContext(nc) as tc:
        with (
            tc.tile_pool(name="const", bufs=1) as cpool,
            tc.tile_pool(name="E", bufs=2) as epool,
            tc.tile_pool(name="qk", bufs=4) as qkpool,
            tc.tile_pool(name="small", bufs=4) as spool,
            tc.tile_pool(name="y", bufs=4) as ypool,
            tc.tile_pool(name="ps", bufs=1, space="PSUM") as pspool,
        ):
            # ---- persistent SBUF tensors ----
            xT_sb = cpool.tile([128, CT, N], bf16)            # 12KB/part
            wqk_sb = cpool.tile([128, 2, CT, C], bf16)        # 18KB
            wv_sb = cpool.tile([128, CT, C], bf16)            # 9KB
            wp_sb = cpool.tile([128, CT, C], bf16)            # 9KB
            pb_sb = cpool.tile([128, CT], f32)
            eye_sb = cpool.tile([128, 128], bf16)
            v_sb = [cpool.tile([128, H, D + 1], bf16, name=f"v{nt}")
                    for nt in range(NT)]                      # 12.2KB
            on_sb = [cpool.tile([128, NCH, 512], bf16, name=f"on{kt}")
                     for kt in range(CT)]                     # 12KB
            stg_sb = [cpool.tile([128, NCH, 512], bf16, name=f"stg{otp}")
                      for otp in range(CT)]                   # 12KB

            # warmup scratch memset first so PE ramp starts ASAP
            scr = cpool.tile([128, 256], bf16, name="scr")
            nc.vector.memset(scr[:], 1.0)

            # input DMA: everything HWDGE goes on the SYNC queue (SP has no
            # compute - issuing from scalar/vector blocks that engine's SEQ
            # behind the shared HWDGE device). Ordered by first use:
            # qk0 inputs, then pair-1 weights, wv, pairs 2-5, wp. xT rows
            # 1/3 ride the gpsimd SWDGE (separate device) in parallel.
            nc.sync.dma_start(xT_sb[:, 0, 0:512], xT_d[0:128, 0:512])
            nc.sync.dma_start(xT_sb[:, 0, 512:1024], xT_d[0:128, 512:1024])
            for role in range(2):
                nc.sync.dma_start(
                    wqk_sb[:, role, 0, :],
                    wqk_d[role, 0, :, :, :].rearrange("b c d -> b (c d)"))
            nc.gpsimd.dma_start(xT_sb[:, 1, :], xT_d[128:256, :])
            nc.gpsimd.dma_start(xT_sb[:, 3, :], xT_d[384:512, :])
            nc.sync.dma_start(xT_sb[:, 2, :], xT_d[256:384, :])
            nc.sync.dma_start(xT_sb[:, 4, :], xT_d[512:640, :])
            nc.sync.dma_start(xT_sb[:, 5, :], xT_d[640:768, :])
            for role in range(2):
                nc.sync.dma_start(
                    wqk_sb[:, role, 1, :],
                    wqk_d[role, 1, :, :, :].rearrange("b c d -> b (c d)"))
            nc.sync.dma_start(wv_sb[:, :, :],
                              wv_d.rearrange("a b c -> b a c"))
            for role in range(2):
                nc.sync.dma_start(
                    wqk_sb[:, role, 2:CT, :],
                    wqk_d[role, 2:CT, :, :, :]
                    .rearrange("a b c d -> b a (c d)"))
            nc.sync.dma_start(wp_sb[:, :, :],
                              wp_d.rearrange("a b c -> b a c"))
            nc.gpsimd.dma_start(pb_sb[:], pb_d[:])
            nc.gpsimd.dma_start(eye_sb[:], eye_d[:])
            # ones column fused into v (softmax sums emerge as O col 64)
            for nt in range(NT):
                nc.vector.memset(v_sb[nt][:, :, D:D + 1], 1.0)

            def body():
                qks = {}      # (hp, role) -> SBUF tile; ("ps",hp,role) -> psum
                vps = {}
                ps_w = pspool.tile([128, 2, 512], f32, tag="f", name="warm")
                for i in range(8):
                    nc.tensor.matmul(ps_w[:, 0, 0:256], scr[:, 0:128],
                                     scr[:], start=True, stop=True)

                def emit_qk_chunk(hp, role, lo, hi, tag="f",
                                  defer_copy=False):
                    key = ("ps", hp, role)
                    if lo == 0:
                        qks[key] = pspool.tile([128, NCH, 512], f32, tag=tag,
                                               bufs=(2 if tag == "s" else 1),
                                               name=f"qk{hp}r{role}")
                    ps = qks[key]
                    for kt in range(lo, hi):
                        for nch in range(NCH):
                            nc.tensor.matmul(
                                ps[:, nch, :],
                                wqk_sb[:, role, hp, kt * 128:kt * 128 + 128],
                                xT_sb[:, kt, nch * 512:(nch + 1) * 512],
                                start=(kt == 0), stop=(kt == CT - 1),
                            )
                    if hi == CT and not defer_copy:
                        qk_copy(hp, role)

                def qk_copy(hp, role):
                    # PSUM->SBUF copy emitted at a fixed low-DVE-pressure
                    # slot so it never delays a chain-gating norm/otcopy
                    key = ("ps", hp, role)
                    ps = qks[key]
                    t = qkpool.tile([128, NCH, 512], bf16, tag="qk",
                                    name=f"qk{hp}r{role}sb")
                    if hp == 0:
                        # startup: split copies across Act (idle) and DVE
                        for nch in range(NCH):
                            if role == 0:
                                nc.scalar.copy(t[:, nch, :], ps[:, nch, :])
                            else:
                                nc.vector.tensor_copy(t[:, nch, :],
                                                      ps[:, nch, :])
                    else:
                        nc.vector.tensor_copy(t[:], ps[:, :, :])
                    qks[(hp, role)] = t
                    del qks[key]

                def emit_v_chunk(nt, lo, hi):
                    # per-och 2KB psums ride the "o" slots (idle until the
                    # phase-1 O chains, which naturally wait on v's copies)
                    if lo == 0:
                        vps[nt] = [
                            pspool.tile([128, 8, 64], f32, tag="o", bufs=2,
                                        name=f"v{nt}o{och}")
                            for och in range(2)]
                    for kt in range(lo, hi):
                        for och in range(2):
                            nc.tensor.matmul(
                                vps[nt][och][:, 0:6, :],
                                xT_sb[:, kt, nt * 128:(nt + 1) * 128],
                                wv_sb[:, kt, och * 384:(och + 1) * 384],
                                start=(kt == 0), stop=(kt == CT - 1),
                            )
                    if hi == CT:
                        for och in range(2):
                            nc.vector.tensor_copy(
                                v_sb[nt][:, och * 6:(och + 1) * 6, 0:D],
                                vps[nt][och][:, 0:6, :],
                            )
                        del vps[nt]

                fillers = []

                def take_fillers(k):
                    for _ in range(min(k, len(fillers))):
                        fillers.pop(0)()

                def queue_qk_chunks(hp):
                    for role in (0, 1):
                        for kt in range(CT):
                            fillers.append(
                                lambda hp=hp, role=role, kt=kt:
                                emit_qk_chunk(hp, role, kt, kt + 1,
                                              defer_copy=(kt == CT - 1)))

                def queue_v_chunks(nt):
                    fillers.append(lambda: emit_v_chunk(nt, 0, 3))
                    fillers.append(lambda: emit_v_chunk(nt, 3, CT))

                pj = {}

                def stage_proj(otp):
                    nc.vector.tensor_copy(stg_sb[otp][:], pj[otp][:, :, :])
                    del pj[otp]

                def proj_chunk(otp, kts, lo, stop_kt, nchs=(0, 1), tag="f"):
                    if kts[0] == lo:
                        pj[otp] = pspool.tile([128, NCH, 512], f32, tag=tag,
                                              bufs=(2 if tag == "s" else 1),
                                              name=f"pj{otp}k{lo}")
                    for kt in kts:
                        for nch in nchs:
                            nc.tensor.matmul(
                                pj[otp][:, nch, :],
                                wp_sb[:, kt, otp * 128:(otp + 1) * 128],
                                on_sb[kt][:, nch, :],
                                start=(kt == lo), stop=(kt == stop_kt),
                            )

                def queue_proj_prefill(otps, last):
                    # prefill kt 0..last (on[last] must be ready a phase
                    # before the pops land), stage partial to SBUF
                    for otp in otps:
                        for kt in range(last + 1):
                            fillers.append(
                                lambda otp=otp, kt=kt:
                                proj_chunk(otp, [kt], 0, last))
                        fillers.append(lambda otp=otp: stage_proj(otp))

                # ---- startup: eager qk(pair 0) on the S psum slots;
                # v0/v1 go through the filler queue (wv lands late).
                # A warm matmul between chunks fills each DMA-wait gap so
                # the PE p-state ramp never resets ----
                for kt in range(CT):
                    emit_qk_chunk(0, 0, kt, kt + 1, tag="s")
                    emit_qk_chunk(0, 1, kt, kt + 1, tag="s")
                    nc.tensor.matmul(ps_w[:, 1, 0:256], scr[:, 0:128],
                                     scr[:], start=True, stop=True)
                # pair-1 role-0 fills the copy window before phase 0
                for kt in range(CT):
                    emit_qk_chunk(1, 0, kt, kt + 1)

                E_prev = None

                # ---- O machinery (orientation B) ----
                def o_chain(ps_o, h, half, qi, lo=0, hi=NT, E=None):
                    """ps_o[:, qi, 0:65] += E^T @ v over key subs lo..hi-1
                    (accumulation sub-order is free: start at 0, stop at 7)."""
                    nch = half
                    qc = qi
                    ab = h % 2
                    Esrc = E_prev if E is None else E
                    for sub in range(lo, hi):
                        nc.tensor.matmul(
                            ps_o[:, qi, 0:D + 1],
                            Esrc[:, sub, nch, ab, qc * 128:(qc + 1) * 128],
                            v_sb[sub][:, h, :],
                            start=(sub == 0), stop=(sub == NT - 1),
                        )

                def o_norm(ps_o, ob, rec, act=False):
                    """rec = 1/sums, ob[q, qt, d-half] = O * rec (bf16) in a
                    single broadcast-AP multiply. ob is this head's 64-col
                    half of the pair tile."""
                    nc.vector.reciprocal_approx_fast(rec[:, :], ps_o[:, :, D])
                    rb = rec[:, :].unsqueeze(-1).broadcast_to([128, 4, D])
                    nc.vector.tensor_tensor(ob[:, :, :], ps_o[:, :, 0:D],
                                            rb, op=mybir.AluOpType.mult)

                def o_transpose(obp, hp0, half, act=False):
                    """ot = obp^T per qtile via eye matmul: both heads at
                    once (shared q rows) -> full-128-partition on chunk."""
                    ot = pspool.tile([128, 4, 128], f32, tag="o",
                                     bufs=2, name=f"ot{hp0}h{half}")
                    for qi in range(4):
                        nc.tensor.matmul(ot[:, qi, :], obp[:, qi, :],
                                         eye_sb[:], start=True, stop=True)
                    dst = on_sb[hp0][:, half, :]
                    if act:
                        nc.scalar.copy(dst, ot[:, :, :])
                    else:
                        nc.vector.tensor_copy(dst, ot[:, :, :])

                def new_o(h, half):
                    ps_o = pspool.tile([128, 4, 128], f32, tag="o",
                                       bufs=2, name=f"o{h}h{half}")
                    rec = spool.tile([128, 4], f32, tag="rec",
                                     name=f"rec{h}h{half}")
                    return ps_o, rec

                def new_obp(hp0, half):
                    obp = ypool.tile([128, 4, 128], bf16, tag="ob", bufs=4,
                                     name=f"obp{hp0}h{half}")
                    return obp

                for hp in range(HP):
                    q_sb, k_sb = qks[(hp, 0)], qks[(hp, 1)]
                    E_ab = epool.tile([128, NT, NCH, 2, 512], bf16, tag="E",
                                      name=f"E{hp}")
                    if hp == 0:
                        for kt in range(CT):
                            fillers.append(
                                lambda kt=kt:
                                emit_qk_chunk(1, 1, kt, kt + 1,
                                              defer_copy=(kt == CT - 1)))
                        for nt in range(NT):
                            queue_v_chunks(nt)
                    elif hp < HP - 1:
                        queue_qk_chunks(hp + 1)
                        if hp == HP - 2:
                            queue_proj_prefill((0, 1), 2)
                    else:
                        queue_proj_prefill((2, 3, 4, 5), 3)

                    ha, hb = (2 * (hp - 1), 2 * (hp - 1) + 1)
                    st = {}
                    for mt in range(NT):
                        for nch in range(NCH):
                            ps_s = pspool.tile([128, 2, 512], f32, tag="s",
                                               bufs=2, name=f"s{hp}m{mt}n{nch}")
                            c0, c1 = mt // 4, (mt % 4) * 128
                            nc.tensor.matmul(
                                ps_s[:, 0, :],
                                k_sb[0:64, c0, c1:c1 + 128],
                                q_sb[0:64, nch, :], start=True, stop=True,
                            )
                            nc.tensor.matmul(
                                ps_s[:, 1, :],
                                k_sb[64:128, c0, c1:c1 + 128],
                                q_sb[64:128, nch, :], start=True, stop=True,
                            )
                            nc.scalar.activation(
                                E_ab[:, mt, nch, :, :], ps_s[:, :, :],
                                mybir.ActivationFunctionType.Exp, scale=SCALE,
                            )
                        if mt == 2 and 0 < hp < HP - 1:
                            qk_copy(hp + 1, 0)
                        if mt == 6 and hp < HP - 1:
                            qk_copy(hp + 1, 1)
                        if E_prev is not None and hp < HP - 1:
                            # per half: a chains -> b chains -> both norms
                            # (a on DVE, b on Act) into the shared pair tile
                            # -> one paired transpose + one full-width copy
                            if mt == 0:
                                st["olo"] = new_obp(hp - 1, 0)
                                st["alo"] = new_o(ha, 0)
                                o_chain(st["alo"][0], ha, 0, 0)
                                o_chain(st["alo"][0], ha, 0, 1)
                            elif mt == 1:
                                o_chain(st["alo"][0], ha, 0, 2)
                                o_chain(st["alo"][0], ha, 0, 3)
                                o_norm(st["alo"][0], st["olo"][:, :, 0:D],
                                       st["alo"][1])
                            elif mt == 2:
                                st["blo"] = new_o(hb, 0)
                                o_chain(st["blo"][0], hb, 0, 0)
                                o_chain(st["blo"][0], hb, 0, 1)
                            elif mt == 3:
                                o_chain(st["blo"][0], hb, 0, 2)
                                o_chain(st["blo"][0], hb, 0, 3)
                                o_norm(st["blo"][0], st["olo"][:, :, D:2 * D],
                                       st["blo"][1])
                            elif mt == 4:
                                o_transpose(st["olo"], hp - 1, 0)
                                st["ohi"] = new_obp(hp - 1, 1)
                                st["ahi"] = new_o(ha, 1)
                                o_chain(st["ahi"][0], ha, 1, 0)
                                o_chain(st["ahi"][0], ha, 1, 1)
                            elif mt == 5:
                                o_chain(st["ahi"][0], ha, 1, 2)
                                o_chain(st["ahi"][0], ha, 1, 3)
                                o_norm(st["ahi"][0], st["ohi"][:, :, 0:D],
                                       st["ahi"][1])
                            elif mt == 6:
                                st["bhi"] = new_o(hb, 1)
                                o_chain(st["bhi"][0], hb, 1, 0)
                                o_chain(st["bhi"][0], hb, 1, 1)
                            elif mt == 7:
                                o_chain(st["bhi"][0], hb, 1, 2)
                                o_chain(st["bhi"][0], hb, 1, 3)
                                o_norm(st["bhi"][0], st["ohi"][:, :, D:2 * D],
                                       st["bhi"][1])
                        elif E_prev is not None:
                            # phase 5: pair-4 O compressed to mts 0-5 so the
                            # tail accumulators allocate before phase end
                            if mt == 0:
                                st["olo"] = new_obp(hp - 1, 0)
                                st["alo"] = new_o(ha, 0)
                                for qi in range(4):
                                    o_chain(st["alo"][0], ha, 0, qi)
                            elif mt == 1:
                                o_norm(st["alo"][0], st["olo"][:, :, 0:D],
                                       st["alo"][1])
                                st["blo"] = new_o(hb, 0)
                                for qi in range(4):
                                    o_chain(st["blo"][0], hb, 0, qi)
                            elif mt == 2:
                                o_norm(st["blo"][0], st["olo"][:, :, D:2 * D],
                                       st["blo"][1])
                            elif mt == 3:
                                o_transpose(st["olo"], hp - 1, 0)
                                st["ohi"] = new_obp(hp - 1, 1)
                                st["ahi"] = new_o(ha, 1)
                                for qi in range(4):
                                    o_chain(st["ahi"][0], ha, 1, qi)
                            elif mt == 4:
                                o_norm(st["ahi"][0], st["ohi"][:, :, 0:D],
                                       st["ahi"][1])
                                st["bhi"] = new_o(hb, 1)
                                for qi in range(4):
                                    o_chain(st["bhi"][0], hb, 1, qi)
                            elif mt == 5:
                                o_norm(st["bhi"][0], st["ohi"][:, :, D:2 * D],
                                       st["bhi"][1])
                            elif mt == 6:
                                o_transpose(st["ohi"], hp - 1, 1)
                                st["t10"] = new_o(H - 2, 0)
                            elif mt == 7:
                                st["t11"] = new_o(H - 1, 0)
                        nfill = 3
                        take_fillers(nfill)
                    if E_prev is not None and hp < HP - 1:
                        take_fillers(2)
                        o_transpose(st["ohi"], hp - 1, 1)
                    take_fillers(len(fillers))
                    E_prev = E_ab
                    tst = st

                # ---- tail: O(pair 5) + proj kt4-5 + epilogues ----
                ha, hb = H - 2, H - 1

                yts = {}

                def proj_fin_nch(otp, nch, ap=None, dve=False):
                    # two fin flavors, alternated to balance the tail:
                    # Act(psum+bias) + DVE tt-add, or a single DVE stt
                    if otp not in yts:
                        yts[otp] = (
                            ypool.tile([128, NCH, 512], bf16, tag="y1",
                                       name=f"y1_{otp}"),
                            ypool.tile([128, NCH, 512], bf16, tag="yt",
                                       name=f"yt_{otp}"),
                        )
                    y1, yt = yts[otp]
                    src_ap = pj[otp][:, nch, :] if ap is None else ap
                    if dve:
                        nc.vector.scalar_tensor_tensor(
                            yt[:, nch, :], src_ap, pb_sb[:, otp:otp + 1],
                            stg_sb[otp][:, nch, :],
                            op0=mybir.AluOpType.add,
                            op1=mybir.AluOpType.add,
                        )
                    else:
                        nc.scalar.activation(
                            y1[:, nch, :], src_ap,
                            mybir.ActivationFunctionType.Identity,
                            bias=pb_sb[:, otp:otp + 1],
                        )
                        nc.vector.tensor_tensor(
                            yt[:, nch, :], y1[:, nch, :],
                            stg_sb[otp][:, nch, :],
                            op=mybir.AluOpType.add,
                        )
                    eng = nc.gpsimd if nch == 0 else nc.sync
                    eng.dma_start(
                        out_d[otp * 128:(otp + 1) * 128,
                              nch * 512:(nch + 1) * 512],
                        yt[:, nch, :])

                def proj_fin_staged(otp):
                    for nch in range(NCH):
                        proj_fin_nch(otp, nch)

                def proj_tail(otp, nch, tag):
                    # wave-2 proj: kt4-5 straight into a freed small slot,
                    # fin immediately (Act + DVE + DMA)
                    pjn = pspool.tile([128, 512], f32, tag=tag,
                                      bufs=(2 if tag == "o" else 1),
                                      name=f"pjt{otp}n{nch}")
                    for kt in (4, 5):
                        nc.tensor.matmul(
                            pjn[:], wp_sb[:, kt, otp * 128:(otp + 1) * 128],
                            on_sb[kt][:, nch, :],
                            start=(kt == 4), stop=(kt == 5))
                    pj[otp] = pjn
                    proj_fin_nch(otp, nch, ap=pjn[:, :], dve=True)

                # proj psums: otp0/1 -> "s" slots, otp2 -> "f"; wave-2
                # otps on the freed 2KB "o"/"f" slots. t10/t11 lo-halves were
                # pre-accumulated (subs 0-6) during phase-5 mts 6-7.
                t10, t11 = tst["t10"], tst["t11"]
                # both hi-half accumulators share one 4KB "f" tile so their
                # chains run immediately, parallel to the lo-half norm path
                obp_lo = new_obp(HP - 1, 0)
                obp_hi = new_obp(HP - 1, 1)
                thi = pspool.tile([128, 8, 128], f32, tag="f", name="thi")
                rec10h = spool.tile([128, 4], f32, tag="rec", name="rec10h")
                rec11h = spool.tile([128, 4], f32, tag="rec", name="rec11h")
                for qi in range(4):
                    o_chain(t10[0], ha, 0, qi, 0, 8, E=E_prev)
                for qi in range(4):
                    o_chain(t11[0], hb, 0, qi, 0, 8, E=E_prev)
                for qi in range(4):
                    o_chain(thi[:, 0:4, :], ha, 1, qi, E=E_prev)
                for qi in range(4):
                    o_chain(thi[:, 4:8, :], hb, 1, qi, E=E_prev)
                o_norm(t10[0], obp_lo[:, :, 0:D], t10[1])
                o_norm(t11[0], obp_lo[:, :, D:2 * D], t11[1])
                o_norm(thi[:, 0:4, :], obp_hi[:, :, 0:D], rec10h)
                o_norm(thi[:, 4:8, :], obp_hi[:, :, D:2 * D], rec11h)
                o_transpose(obp_lo, HP - 1, 0)
                proj_chunk(0, [3], 3, 5, tag="s")
                o_transpose(obp_hi, HP - 1, 1, act=True)
                proj_chunk(1, [3], 3, 5, tag="s")
                proj_chunk(0, [4], 3, 5)
                proj_chunk(1, [4], 3, 5)
                proj_chunk(2, [4], 4, 5, tag="f")
                # on5 nch0 complete: kt5 nch0 for otp0-2, fin eagerly
                proj_chunk(0, [5], 3, 5, nchs=(0,))
                proj_fin_nch(0, 0, dve=True)
                proj_chunk(1, [5], 3, 5, nchs=(0,))
                proj_fin_nch(1, 0, dve=True)
                proj_chunk(2, [5], 4, 5, nchs=(0,))
                proj_fin_nch(2, 0, dve=True)
                proj_tail(3, 0, "o")
                # wave-2: otp3 on the "o" slots, otp4/5 on the freed "s"
                # slots (allocated only after pj0/pj1 fully drain them)
                proj_chunk(0, [5], 3, 5, nchs=(1,))
                proj_fin_nch(0, 1)
                proj_chunk(4, [4], 4, 5, tag="s")
                proj_tail(3, 1, "o")
                proj_chunk(1, [5], 3, 5, nchs=(1,))
                proj_fin_nch(1, 1, dve=True)
                proj_chunk(5, [4], 4, 5, tag="s")
                proj_chunk(4, [5], 4, 5, nchs=(0,))
                proj_fin_nch(4, 0)
                proj_chunk(2, [5], 4, 5, nchs=(1,))
                proj_fin_nch(2, 1)
                proj_chunk(5, [5], 4, 5, nchs=(0,))
                proj_fin_nch(5, 0)
                proj_chunk(4, [5], 4, 5, nchs=(1,))
                proj_fin_nch(4, 1)
                proj_chunk(5, [5], 4, 5, nchs=(1,))
                proj_fin_nch(5, 1)

            if loop_r is not None:
                with tc.For_i(0, loop_r):
                    body()
            else:
                body()

    nc.compile()
    return nc


def _get_nc():
    if "nc" not in _CACHE:
        _CACHE["nc"] = _build_nc()
    return _CACHE["nc"]


def kernel(x, qkv_w, proj_w, proj_b):
    from concourse.bass_utils import run_bass_kernel_spmd

    nc = _get_nc()
    bf = ml_dtypes.bfloat16
    wqk = np.ascontiguousarray(
        qkv_w[:2 * C].T.reshape(CT, 128, 2, CT, 128)
        .transpose(2, 3, 1, 0, 4)).astype(bf)
    wv = np.ascontiguousarray(qkv_w[2 * C:].T).astype(bf).reshape(CT, 128, C)
    wp = np.ascontiguousarray(proj_w.T).astype(bf).reshape(CT, 128, C)
    pb = np.ascontiguousarray(proj_b.reshape(CT, 128).T).astype(np.float32)
    eye = np.eye(128, dtype=bf)
    in_maps = []
    for i in range(B):
        in_maps.append({
            "xT": np.ascontiguousarray(x[i].T).astype(bf),
            "wqk": wqk, "wv": wv, "wp": wp, "pb": pb, "eye": eye,
        })
    res = run_bass_kernel_spmd(nc, in_maps, core_ids=list(range(B)))
    out = np.stack([res.results[i]["out"].astype(np.float32).T for i in range(B)])
    return np.ascontiguousarray(out)
